# revision 1
# baseline (speedup 1.0000x reference)
"""Trainium2 Bass kernel for nn_MixingNetwork (QMIX-style mixer).

Math (per sample b):
  hid_w = (state @ W_hw).reshape(H, N); out_w = state @ W_ow; hid_b = state @ W_hb
  h     = elu(hid_w @ q + hid_b);      q_tot = out_w . h + state @ W_ob (+ biases)

Strategy: pure data parallel over batch (512 samples/core on 8 cores).
The dominant contraction state @ W_hw runs in bf16 on the PE. W_hw columns
are host-permuted to n-outer order (col = n*H + h) so the per-sample
q-weighting becomes a per-partition-scalar multiply on the Scalar engine
(activation Copy with scale=q[:, n]), and the sum over the 64 agents is a
strided tensor_reduce on the Vector engine, grouped to pipeline with the PE.
All biases are folded in as rank-1 / small matmuls accumulated in PSUM.
"""

import numpy as np
import ml_dtypes

B, N_AGENTS, HIDDEN, STATE_DIM = 4096, 64, 256, 512
N_CORES = 8
BS = B // N_CORES          # samples per core
NBT = BS // 128            # b-tiles per core
KT = STATE_DIM // 128      # k-tiles over state dim
FCHUNK = 512               # columns of W per PSUM chunk
NF = HIDDEN * N_AGENTS // FCHUNK   # 32 chunks
NPC = FCHUNK // HIDDEN     # agents (n) per chunk = 2
GROUP = 8                  # f-chunks per partial-reduce group
NG = NF // GROUP           # groups

_CACHE = {}

# build-time tuning knobs (A/B testing)
CFG = {
    "dve_every": 4,      # every Nth scale op on DVE (0 = all on ACT)
    "sync_w_dma": True,  # W-chunk DMAs via HWDGE (sync engine)
    "ps_bufs": 4,        # big-psum pool bufs
    "w_bufs": 6,
    "mode": "full",     # "dma" | "mm" | "full" — partial builds for HW bisect
    "arch": "v2",       # v1: scale-from-PSUM + reduce; v2: pre-scaled movers
    "mv_bufs": 24,
    "qrep_sync": True,  # qrep/const broadcast DMAs via HWDGE
    "mv_pair": True,    # one DVE op builds movers for both agents of a chunk
}


def _emit_body(nc, tc, ctx, tile, mybir, dram, pools):
    bass = pools["bass"]
    bf16 = mybir.dt.bfloat16
    f32 = mybir.dt.float32
    AX = mybir.AxisListType
    ALU = mybir.AluOpType
    ACTF = mybir.ActivationFunctionType

    stateT, q, qT, w_perm, w_small, bmat, bias_small, out = dram
    consts, wpool, spool, papool, hpool, pspool, smpool = (
        pools["consts"], pools["w"], pools["s"], pools["pa"], pools["h"],
        pools["ps"], pools["sm"],
    )

    # ---- constants into SBUF (emitted per rep; cheap) ----
    stateT_sb = consts.tile([128, KT, BS], bf16, tag="stateT")
    nc.sync.dma_start(stateT_sb[:], stateT.rearrange("(k p) b -> p k b", p=128))
    q_sb = consts.tile([128, NBT, N_AGENTS], f32, tag="q")
    nc.gpsimd.dma_start(q_sb[:], q.rearrange("(t p) n -> p t n", p=128))
    wsm_sb = consts.tile([128, KT, HIDDEN * 2 + 2], bf16, tag="wsm")
    nc.gpsimd.dma_start(wsm_sb[:], w_small.rearrange("(k p) c -> p k c", p=128))
    qT_sb = consts.tile([N_AGENTS, BS], bf16, tag="qT")
    nc.gpsimd.dma_start(qT_sb[:], qT[:, :])
    bmat_sb = consts.tile([N_AGENTS, HIDDEN], bf16, tag="bmat")
    nc.gpsimd.dma_start(bmat_sb[:], bmat[:, :])
    bias_sb = consts.tile([1, HIDDEN * 2 + 2], bf16, tag="bias")
    nc.gpsimd.dma_start(bias_sb[:], bias_small[:, :])
    ones_sb = consts.tile([1, 128], bf16, tag="ones")
    nc.vector.memset(ones_sb[:], 1.0)

    partials = [papool.tile([128, NG, HIDDEN], f32, tag=f"pa{bt}", name=f"pa{bt}")
                for bt in range(NBT)]
    S = [None] * NBT

    # ---- big contraction: G = stateT.T @ W_perm, scaled by q, reduced over n ----
    # fraction of scale ops routed to DVE (rest on ACT) to balance engines
    ndve = 0
    for f in range(NF):
        wt = wpool.tile([128, KT, FCHUNK], bf16, tag="w")
        dma_eng = nc.sync if CFG["sync_w_dma"] else nc.gpsimd
        dma_eng.dma_start(
            wt[:],
            w_perm.rearrange("(k p) n -> p k n", p=128)[:, :, f * FCHUNK:(f + 1) * FCHUNK],
        )
        g, pos = divmod(f, GROUP)
        for bt in range(NBT):
            if pos == 0:
                S[bt] = spool.tile([128, NPC * GROUP, HIDDEN], bf16, tag=f"S{bt}",
                                   name=f"S{bt}")
            if CFG["mode"] == "dma":
                continue
            ps = pspool.tile([128, FCHUNK], f32, tag="ps")
            bsl = slice(bt * 128, (bt + 1) * 128)
            for k in range(KT):
                nc.tensor.matmul(
                    ps[:], stateT_sb[:, k, bsl], wt[:, k, :],
                    start=(k == 0), stop=(k == KT - 1),
                )
            for j in range(NPC if CFG["mode"] == "full" else 0):
                n_local = NPC * pos + j
                n_glob = NPC * f + j
                ndve += 1
                if CFG["dve_every"] and ndve % CFG["dve_every"] == 0:
                    nc.vector.tensor_scalar_mul(
                        S[bt][:, n_local, :], ps[:, j * HIDDEN:(j + 1) * HIDDEN],
                        q_sb[:, bt, n_glob:n_glob + 1],
                    )
                else:
                    nc.scalar.activation(
                        S[bt][:, n_local, :], ps[:, j * HIDDEN:(j + 1) * HIDDEN],
                        ACTF.Copy, scale=q_sb[:, bt, n_glob:n_glob + 1],
                    )
            if pos == GROUP - 1 and CFG["mode"] == "full":
                nc.vector.tensor_reduce(
                    partials[bt][:, g, :],
                    S[bt][:].rearrange("p n h -> p h n"),
                    axis=AX.X, op=ALU.add,
                )

    # ---- per-b-tile tail: hypernet biases + small matmuls + ELU + final dot ----
    for bt in range(NBT if CFG["mode"] == "full" else 0):
        bsl = slice(bt * 128, (bt + 1) * 128)
        hs = smpool.tile([128, HIDDEN], f32, tag="hsum")
        for k in range(KT):
            nc.tensor.matmul(hs[:], stateT_sb[:, k, bsl], wsm_sb[:, k, 0:HIDDEN],
                             start=(k == 0), stop=False)
        nc.tensor.matmul(hs[:], qT_sb[:, bsl], bmat_sb[:], start=False, stop=False)
        nc.tensor.matmul(hs[:], ones_sb[:, 0:128], bias_sb[:, 0:HIDDEN],
                         start=False, stop=True)

        ow = smpool.tile([128, HIDDEN], f32, tag="ow")
        for k in range(KT):
            nc.tensor.matmul(ow[:], stateT_sb[:, k, bsl],
                             wsm_sb[:, k, HIDDEN:2 * HIDDEN],
                             start=(k == 0), stop=False)
        nc.tensor.matmul(ow[:], ones_sb[:, 0:128], bias_sb[:, HIDDEN:2 * HIDDEN],
                         start=False, stop=True)

        ob = smpool.tile([128, 1], f32, tag="ob")
        for k in range(KT):
            nc.tensor.matmul(ob[:], stateT_sb[:, k, bsl],
                             wsm_sb[:, k, 2 * HIDDEN:2 * HIDDEN + 1],
                             start=(k == 0), stop=False)
        nc.tensor.matmul(ob[:], ones_sb[:, 0:128], bias_sb[:, 2 * HIDDEN:2 * HIDDEN + 1],
                         start=False, stop=True)

        hpre = hpool.tile([128, HIDDEN], f32, tag="hpre")
        nc.vector.tensor_reduce(
            hpre[:], partials[bt][:].rearrange("p g h -> p h g"), axis=AX.X, op=ALU.add,
        )
        hp = hpool.tile([128, HIDDEN], f32, tag="hp")
        nc.vector.tensor_add(hp[:], hpre[:], hs[:])
        # elu(x) = max(x,0) + exp(min(x,0)) - 1
        t0 = hpool.tile([128, HIDDEN], f32, tag="t0")
        nc.vector.tensor_scalar_min(t0[:], hp[:], 0.0)
        ex = hpool.tile([128, HIDDEN], f32, tag="ex")
        nc.scalar.activation(ex[:], t0[:], ACTF.Exp)
        t1 = hpool.tile([128, HIDDEN], f32, tag="t1")
        nc.vector.tensor_scalar_max(t1[:], hp[:], 0.0)
        h2 = hpool.tile([128, HIDDEN], f32, tag="h2")
        nc.vector.tensor_add(h2[:], t1[:], ex[:])
        h3 = hpool.tile([128, HIDDEN], f32, tag="h3")
        nc.vector.tensor_scalar_add(h3[:], h2[:], -1.0)

        scr = hpool.tile([128, HIDDEN], f32, tag="scr")
        nc.vector.tensor_mul(scr[:], h3[:], ow[:])
        qts = hpool.tile([128, 1], f32, tag="qts")
        nc.vector.tensor_reduce(qts[:], scr[:], axis=AX.X, op=ALU.add)
        qt = hpool.tile([128, 1], f32, tag="qt")
        nc.vector.tensor_add(qt[:], qts[:], ob[:, 0:1])
        nc.gpsimd.dma_start(out[bsl, :], qt[:])


def _emit_body_v2(nc, tc, ctx, tile, mybir, dram, pools):
    """[h, b]-oriented pipeline: W stationary, movers = stateT * q[b, n] built
    on DVE (bf16 SBUF 2x); agent-sum accumulates in PSUM; final dot = ones
    matmul over h partitions. ACT nearly idle; PE-bound."""
    import concourse.bass as bass
    bf16 = mybir.dt.bfloat16
    f32 = mybir.dt.float32
    AX = mybir.AxisListType
    ALU = mybir.AluOpType
    ACTF = mybir.ActivationFunctionType
    H2 = HIDDEN // 128              # h-tiles (2)

    stateT, q, qT, w_perm, w_small, bmat, bias_small, out = dram
    consts, wpool, mvpool, hpool, pspool = (
        pools["consts"], pools["w"], pools["mv"], pools["h"], pools["ps"])

    dma = (nc.sync if CFG["sync_w_dma"] else nc.gpsimd).dma_start
    qdma = (nc.sync if CFG["qrep_sync"] else nc.gpsimd).dma_start

    # ---- constants; stateT split per k-tile so the first slice lands fast
    stateT_sb = consts.tile([128, KT, BS], bf16, tag="stateT")
    stateT_r = stateT.rearrange("(k p) b -> p k b", p=128)
    for k in range(KT):
        dma(stateT_sb[:, k, :], stateT_r[:, k, :])
    wsm_sb = consts.tile([128, KT, HIDDEN * 2 + 2], bf16, tag="wsm")
    nc.gpsimd.dma_start(wsm_sb[:], w_small.rearrange("(k p) c -> p k c", p=128))
    qT_sb = consts.tile([N_AGENTS, BS], bf16, tag="qT")
    nc.gpsimd.dma_start(qT_sb[:], qT[:, :])
    bmat_sb = consts.tile([N_AGENTS, HIDDEN], bf16, tag="bmat")
    nc.gpsimd.dma_start(bmat_sb[:], bmat[:, :])
    bias_sb = consts.tile([1, HIDDEN * 2 + 2], bf16, tag="bias")
    nc.gpsimd.dma_start(bias_sb[:], bias_small[:, :])
    ones_row = consts.tile([1, BS], bf16, tag="ones_row")
    nc.vector.memset(ones_row[:], 1.0)
    ones_col = consts.tile([128, 1], bf16, tag="ones_col")
    nc.vector.memset(ones_col[:], 1.0)

    NQG = 8
    NQTILES = N_AGENTS // NQG
    qrep = [consts.tile([128, NQG, BS], bf16, tag=f"qrep{g}", name=f"qrep{g}")
            for g in range(NQTILES)]

    Y = [pspool.tile([128, BS], f32, tag=f"Y{ht}", name=f"Y{ht}") for ht in range(H2)]
    OW = [pspool.tile([128, BS], f32, tag=f"OW{ht}", name=f"OW{ht}") for ht in range(H2)]
    QP = pspool.tile([1, BS], f32, tag="QP")

    # ---- small matmuls first: they run during the DMA ramp-up while the
    # first W chunks stream in. Y accumulation group OPENS here (start=True)
    # and is closed by the last big-loop matmul.
    for ht in range(H2):
        hsl = slice(ht * 128, (ht + 1) * 128)
        for k in range(KT):
            nc.tensor.matmul(Y[ht][:], wsm_sb[:, k, hsl], stateT_sb[:, k, :],
                             start=(k == 0), stop=False, skip_group_check=True)
        nc.tensor.matmul(Y[ht][:], bmat_sb[:, hsl], qT_sb[:, :],
                         start=False, stop=False, skip_group_check=True)
        nc.tensor.matmul(Y[ht][:], bias_sb[:, hsl], ones_row[:, :],
                         start=False, stop=False, skip_group_check=True)
        for k in range(KT):
            nc.tensor.matmul(OW[ht][:], wsm_sb[:, k, HIDDEN + ht * 128:HIDDEN + (ht + 1) * 128],
                             stateT_sb[:, k, :],
                             start=(k == 0), stop=False, skip_group_check=True)
        nc.tensor.matmul(OW[ht][:], bias_sb[:, HIDDEN + ht * 128:HIDDEN + (ht + 1) * 128],
                         ones_row[:, :], start=False, stop=True, skip_group_check=True)
    for k in range(KT):
        nc.tensor.matmul(QP[:], wsm_sb[:, k, 2 * HIDDEN:2 * HIDDEN + 1],
                         stateT_sb[:, k, :], start=(k == 0), stop=False,
                         skip_group_check=True)
    nc.tensor.matmul(QP[:], bias_sb[:, 2 * HIDDEN:2 * HIDDEN + 1], ones_row[:, :],
                     start=False, stop=False, skip_group_check=True)
    # q_tot -= sum_h out_w[b, h]  (compensates using elu+1 in the tail)
    for k in range(KT):
        nc.tensor.matmul(QP[:], wsm_sb[:, k, 2 * HIDDEN + 1:2 * HIDDEN + 2],
                         stateT_sb[:, k, :], start=False, stop=False,
                         skip_group_check=True)
    nc.tensor.matmul(QP[:], bias_sb[:, 2 * HIDDEN + 1:2 * HIDDEN + 2], ones_row[:, :],
                     start=False, stop=False, skip_group_check=True)

    # ---- big streamed contraction; qrep slices interleave with the W stream
    w_perm_r = w_perm.rearrange("(k p) n -> p k n", p=128)
    for f in range(NF):
        if f % (NF // NQTILES) == 0:
            g = f // (NF // NQTILES)
            qsrc = bass.AP(
                tensor=qT.tensor, offset=g * NQG * BS,
                ap=[[0, 128], [BS, NQG], [1, BS]],
            )
            qdma(qrep[g][:], qsrc)
        wt = wpool.tile([128, KT, FCHUNK], bf16, tag="w")
        dma(wt[:], w_perm_r[:, :, f * FCHUNK:(f + 1) * FCHUNK])
        n0 = NPC * f
        for k in range(KT):
            last = (f == NF - 1 and k == KT - 1)
            mv2 = mvpool.tile([128, NPC, BS], bf16, tag="mv")
            st_rep = bass.AP(
                tensor=stateT_sb.tensor, offset=stateT_sb[:, k, :].offset,
                ap=[stateT_sb[:].ap[0], [0, NPC], [1, BS]],
            )
            nc.vector.tensor_mul(mv2[:], st_rep,
                                 qrep[n0 // NQG][:, n0 % NQG:n0 % NQG + NPC, :])
            for j in range(NPC):
                for ht in range(H2):
                    nc.tensor.matmul(
                        Y[ht][:], wt[:, k, j * HIDDEN + ht * 128:j * HIDDEN + (ht + 1) * 128],
                        mv2[:, j, :], start=False,
                        stop=last and j == NPC - 1, skip_group_check=True,
                    )

    # ---- tail: elu + dot with out_w^T, h-reduction via ones matmul.
    # Split along b so the chain pipelines in smaller quanta.
    NBH = 1
    BH = BS // NBH
    for ht in range(H2):
        for hb in range(NBH):
            bsl = slice(hb * BH, (hb + 1) * BH)
            t0 = hpool.tile([128, BH], f32, tag="t0")
            nc.vector.tensor_scalar_min(t0[:], Y[ht][:, bsl], 0.0)
            ex = hpool.tile([128, BH], f32, tag="ex")
            nc.scalar.activation(ex[:], t0[:], ACTF.Exp)
            rl = hpool.tile([128, BH], f32, tag="rl")
            nc.scalar.activation(rl[:], Y[ht][:, bsl], ACTF.Relu)
            h3 = hpool.tile([128, BH], f32, tag="h3")
            nc.vector.tensor_add(h3[:], rl[:], ex[:])
            mT = hpool.tile([128, BH], bf16, tag="mT")
            nc.vector.tensor_mul(mT[:], h3[:], OW[ht][:, bsl])
            nc.tensor.matmul(QP[:, bsl], ones_col[:, 0:1], mT[:],
                             start=False, stop=(ht == H2 - 1 and hb == NBH - 1),
                             skip_group_check=True)
    qrow = hpool.tile([1, BS], f32, tag="qrow")
    nc.scalar.activation(qrow[:], QP[:], ACTF.Copy)
    nc.gpsimd.dma_start(out[:, :], qrow[:])


def build_module(reps=1, loop_reps=1):
    """Build and compile the per-core Bass module. reps>1 repeats the whole
    computation in one NEFF (for timing)."""
    from contextlib import ExitStack
    import concourse.bass as bass
    import concourse.tile as tile
    from concourse import bacc, mybir

    bf16 = mybir.dt.bfloat16
    f32 = mybir.dt.float32

    nc = bacc.Bacc("TRN2", target_bir_lowering=False)
    stateT = nc.dram_tensor("stateT", [STATE_DIM, BS], bf16, kind="ExternalInput").ap()
    q = nc.dram_tensor("q", [BS, N_AGENTS], f32, kind="ExternalInput").ap()
    qT = nc.dram_tensor("qT", [N_AGENTS, BS], bf16, kind="ExternalInput").ap()
    w_perm = nc.dram_tensor("w_perm", [STATE_DIM, HIDDEN * N_AGENTS], bf16,
                            kind="ExternalInput").ap()
    w_small = nc.dram_tensor("w_small", [STATE_DIM, HIDDEN * 2 + 2], bf16,
                             kind="ExternalInput").ap()
    bmat = nc.dram_tensor("bmat", [N_AGENTS, HIDDEN], bf16, kind="ExternalInput").ap()
    bias_small = nc.dram_tensor("bias_small", [1, HIDDEN * 2 + 2], bf16,
                                kind="ExternalInput").ap()
    if CFG["arch"] == "v2":
        out = nc.dram_tensor("out", [1, BS], f32, kind="ExternalOutput").ap()
    else:
        out = nc.dram_tensor("out", [BS, 1], f32, kind="ExternalOutput").ap()
    dram = (stateT, q, qT, w_perm, w_small, bmat, bias_small, out)

    with tile.TileContext(nc) as tc:
        with ExitStack() as ctx:
            if CFG["arch"] == "v2":
                pools = {
                    "bass": bass,
                    "consts": ctx.enter_context(tc.tile_pool(name="consts", bufs=1)),
                    "w": ctx.enter_context(tc.tile_pool(name="w", bufs=CFG["w_bufs"])),
                    "mv": ctx.enter_context(tc.tile_pool(name="mv", bufs=CFG["mv_bufs"])),
                    "h": ctx.enter_context(tc.tile_pool(name="h", bufs=2)),
                    "ps": ctx.enter_context(tc.tile_pool(name="ps", bufs=1, space="PSUM")),
                }
                emit = _emit_body_v2
            else:
                pools = {
                    "bass": bass,
                    "consts": ctx.enter_context(tc.tile_pool(name="consts", bufs=1)),
                    "w": ctx.enter_context(tc.tile_pool(name="w", bufs=CFG["w_bufs"])),
                    "s": ctx.enter_context(tc.tile_pool(name="s", bufs=2)),
                    "pa": ctx.enter_context(tc.tile_pool(name="pa", bufs=1)),
                    "h": ctx.enter_context(tc.tile_pool(name="h", bufs=2)),
                    "ps": ctx.enter_context(tc.tile_pool(name="ps", bufs=CFG["ps_bufs"], space="PSUM")),
                    "sm": ctx.enter_context(tc.tile_pool(name="sm", bufs=1, space="PSUM")),
                }
                emit = _emit_body
            if loop_reps > 1:
                with tc.For_i(0, loop_reps, 1,
                              hint_engines=(mybir.EngineType.PE,
                                            mybir.EngineType.DVE)):
                    for _ in range(reps):
                        emit(nc, tc, ctx, tile, mybir, dram, pools)
            else:
                for _ in range(reps):
                    emit(nc, tc, ctx, tile, mybir, dram, pools)
    nc.compile()
    return nc


def make_in_maps(q_values, state_representation, W_hw, b_hw, W_ow, b_ow, W_hb,
                 b_hb, W_ob, b_ob):
    bf16 = ml_dtypes.bfloat16
    q = np.asarray(q_values, dtype=np.float32).reshape(B, N_AGENTS)
    st = np.asarray(state_representation, dtype=np.float32)
    W_hw = np.asarray(W_hw, dtype=np.float32)
    # permute columns of W_hw from (h, n) to (n, h) order
    w_perm = np.ascontiguousarray(
        W_hw.reshape(STATE_DIM, HIDDEN, N_AGENTS).transpose(0, 2, 1)
        .reshape(STATE_DIM, HIDDEN * N_AGENTS)).astype(bf16)
    W_ow32 = np.asarray(W_ow, np.float32)
    w_small = np.ascontiguousarray(np.concatenate(
        [np.asarray(W_hb, np.float32), W_ow32,
         np.asarray(W_ob, np.float32),
         -W_ow32.sum(axis=1, keepdims=True)], axis=1)).astype(bf16)
    bmat = np.ascontiguousarray(
        np.asarray(b_hw, np.float32).reshape(HIDDEN, N_AGENTS).T).astype(bf16)
    b_ow32 = np.asarray(b_ow, np.float32)
    bias_small = np.concatenate(
        [np.asarray(b_hb, np.float32), b_ow32,
         np.asarray(b_ob, np.float32),
         -b_ow32.sum(keepdims=True)]).reshape(1, HIDDEN * 2 + 2).astype(bf16)
    in_maps = []
    for c in range(N_CORES):
        sl = slice(c * BS, (c + 1) * BS)
        in_maps.append({
            "stateT": np.ascontiguousarray(st[sl].T).astype(bf16),
            "q": np.ascontiguousarray(q[sl]),
            "qT": np.ascontiguousarray(q[sl].T).astype(bf16),
            "w_perm": w_perm,
            "w_small": w_small,
            "bmat": bmat,
            "bias_small": bias_small,
        })
    return in_maps


def kernel(**inputs):
    from concourse.bass_utils import run_bass_kernel_spmd

    if "nc" not in _CACHE:
        _CACHE["nc"] = build_module()
    nc = _CACHE["nc"]
    in_maps = make_in_maps(**inputs)
    res = run_bass_kernel_spmd(nc, in_maps, core_ids=list(range(N_CORES)))
    if CFG["arch"] == "v2":
        out = np.concatenate(
            [res.results[c]["out"][0] for c in range(N_CORES)]).reshape(B, 1)
    else:
        out = np.concatenate([res.results[c]["out"] for c in range(N_CORES)], axis=0)
    return out.astype(np.float32)



# revision 19
# speedup vs baseline: 175.8363x; 175.8363x over previous
"""Trainium2 Bass kernel for nn_MixingNetwork (QMIX-style mixer).

Math (per sample b):
  hid_w = (state @ W_hw).reshape(H, N); out_w = state @ W_ow; hid_b = state @ W_hb
  h     = elu(hid_w @ q + hid_b);      q_tot = out_w . h + state @ W_ob (+ biases)

Strategy: pure data parallel over batch (512 samples/core on 8 cores).
The dominant contraction state @ W_hw runs in bf16 on the PE. W_hw columns
are host-permuted to n-outer order (col = n*H + h) so the per-sample
q-weighting becomes a per-partition-scalar multiply on the Scalar engine
(activation Copy with scale=q[:, n]), and the sum over the 64 agents is a
strided tensor_reduce on the Vector engine, grouped to pipeline with the PE.
All biases are folded in as rank-1 / small matmuls accumulated in PSUM.
"""

import numpy as np
import ml_dtypes

B, N_AGENTS, HIDDEN, STATE_DIM = 4096, 64, 256, 512
N_CORES = 8
BS = B // N_CORES          # samples per core
NBT = BS // 128            # b-tiles per core
KT = STATE_DIM // 128      # k-tiles over state dim
FCHUNK = 512               # columns of W per PSUM chunk
NF = HIDDEN * N_AGENTS // FCHUNK   # 32 chunks
NPC = FCHUNK // HIDDEN     # agents (n) per chunk = 2
GROUP = 8                  # f-chunks per partial-reduce group
NG = NF // GROUP           # groups

_CACHE = {}

# build-time tuning knobs (A/B testing)
CFG = {
    "dve_every": 4,      # every Nth scale op on DVE (0 = all on ACT)
    "sync_w_dma": True,  # W-chunk DMAs via HWDGE (sync engine)
    "ps_bufs": 4,        # big-psum pool bufs
    "w_bufs": 6,
    "mode": "full",     # "dma" | "mm" | "full" — partial builds for HW bisect
    "arch": "v2",       # v1: scale-from-PSUM + reduce; v2: pre-scaled movers
    "mv_bufs": 24,
    "qrep_sync": True,  # qrep/const broadcast DMAs via HWDGE
    "mv_pair": True,    # one DVE op builds movers for both agents of a chunk
    "fp8k": 1,          # of the 4 k-tiles, how many (from the top) run as
                        # fp8e4m3 DoubleRow (2 agents/MM). err ~1.7e-2 at 1.
    "fp8_s": 4.0,       # balance scale: W*s on host, state/s on device
    "mv8_via": "act",   # "act": DVE builds bf16 mover, ACT casts to fp8
                        # (keeps DVE at its bf16 rate); "dve": direct fp8 TT
    "korder": (0, 3, 1, 2),  # emission order of k-tiles within an f-chunk:
                        # DR matmuls mid-chunk hide their longer LDWEIGHTS
    "w8_gp": False,     # w8t DMAs on the Pool queue (measured: neutral)
    "swil": False,      # DoubleRowSwInterleave: host pre-interleaves W pairs
                        # (contiguous LDWEIGHTS, no HW reversal penalty)
}


def _emit_body(nc, tc, ctx, tile, mybir, dram, pools):
    bass = pools["bass"]
    bf16 = mybir.dt.bfloat16
    f32 = mybir.dt.float32
    AX = mybir.AxisListType
    ALU = mybir.AluOpType
    ACTF = mybir.ActivationFunctionType

    stateT, q, qT, w_perm, w_small, bmat, bias_small, out = dram
    consts, wpool, spool, papool, hpool, pspool, smpool = (
        pools["consts"], pools["w"], pools["s"], pools["pa"], pools["h"],
        pools["ps"], pools["sm"],
    )

    # ---- constants into SBUF (emitted per rep; cheap) ----
    stateT_sb = consts.tile([128, KT, BS], bf16, tag="stateT")
    nc.sync.dma_start(stateT_sb[:], stateT.rearrange("(k p) b -> p k b", p=128))
    q_sb = consts.tile([128, NBT, N_AGENTS], f32, tag="q")
    nc.gpsimd.dma_start(q_sb[:], q.rearrange("(t p) n -> p t n", p=128))
    wsm_sb = consts.tile([128, KT, HIDDEN * 2 + 2], bf16, tag="wsm")
    nc.gpsimd.dma_start(wsm_sb[:], w_small.rearrange("(k p) c -> p k c", p=128))
    qT_sb = consts.tile([N_AGENTS, BS], bf16, tag="qT")
    nc.gpsimd.dma_start(qT_sb[:], qT[:, :])
    bmat_sb = consts.tile([N_AGENTS, HIDDEN], bf16, tag="bmat")
    nc.gpsimd.dma_start(bmat_sb[:], bmat[:, :])
    bias_sb = consts.tile([1, HIDDEN * 2 + 2], bf16, tag="bias")
    nc.gpsimd.dma_start(bias_sb[:], bias_small[:, :])
    ones_sb = consts.tile([1, 128], bf16, tag="ones")
    nc.vector.memset(ones_sb[:], 1.0)

    partials = [papool.tile([128, NG, HIDDEN], f32, tag=f"pa{bt}", name=f"pa{bt}")
                for bt in range(NBT)]
    S = [None] * NBT

    # ---- big contraction: G = stateT.T @ W_perm, scaled by q, reduced over n ----
    # fraction of scale ops routed to DVE (rest on ACT) to balance engines
    ndve = 0
    for f in range(NF):
        wt = wpool.tile([128, KT, FCHUNK], bf16, tag="w")
        dma_eng = nc.sync if CFG["sync_w_dma"] else nc.gpsimd
        dma_eng.dma_start(
            wt[:],
            w_perm.rearrange("(k p) n -> p k n", p=128)[:, :, f * FCHUNK:(f + 1) * FCHUNK],
        )
        g, pos = divmod(f, GROUP)
        for bt in range(NBT):
            if pos == 0:
                S[bt] = spool.tile([128, NPC * GROUP, HIDDEN], bf16, tag=f"S{bt}",
                                   name=f"S{bt}")
            if CFG["mode"] == "dma":
                continue
            ps = pspool.tile([128, FCHUNK], f32, tag="ps")
            bsl = slice(bt * 128, (bt + 1) * 128)
            for k in range(KT):
                nc.tensor.matmul(
                    ps[:], stateT_sb[:, k, bsl], wt[:, k, :],
                    start=(k == 0), stop=(k == KT - 1),
                )
            for j in range(NPC if CFG["mode"] == "full" else 0):
                n_local = NPC * pos + j
                n_glob = NPC * f + j
                ndve += 1
                if CFG["dve_every"] and ndve % CFG["dve_every"] == 0:
                    nc.vector.tensor_scalar_mul(
                        S[bt][:, n_local, :], ps[:, j * HIDDEN:(j + 1) * HIDDEN],
                        q_sb[:, bt, n_glob:n_glob + 1],
                    )
                else:
                    nc.scalar.activation(
                        S[bt][:, n_local, :], ps[:, j * HIDDEN:(j + 1) * HIDDEN],
                        ACTF.Copy, scale=q_sb[:, bt, n_glob:n_glob + 1],
                    )
            if pos == GROUP - 1 and CFG["mode"] == "full":
                nc.vector.tensor_reduce(
                    partials[bt][:, g, :],
                    S[bt][:].rearrange("p n h -> p h n"),
                    axis=AX.X, op=ALU.add,
                )

    # ---- per-b-tile tail: hypernet biases + small matmuls + ELU + final dot ----
    for bt in range(NBT if CFG["mode"] == "full" else 0):
        bsl = slice(bt * 128, (bt + 1) * 128)
        hs = smpool.tile([128, HIDDEN], f32, tag="hsum")
        for k in range(KT):
            nc.tensor.matmul(hs[:], stateT_sb[:, k, bsl], wsm_sb[:, k, 0:HIDDEN],
                             start=(k == 0), stop=False)
        nc.tensor.matmul(hs[:], qT_sb[:, bsl], bmat_sb[:], start=False, stop=False)
        nc.tensor.matmul(hs[:], ones_sb[:, 0:128], bias_sb[:, 0:HIDDEN],
                         start=False, stop=True)

        ow = smpool.tile([128, HIDDEN], f32, tag="ow")
        for k in range(KT):
            nc.tensor.matmul(ow[:], stateT_sb[:, k, bsl],
                             wsm_sb[:, k, HIDDEN:2 * HIDDEN],
                             start=(k == 0), stop=False)
        nc.tensor.matmul(ow[:], ones_sb[:, 0:128], bias_sb[:, HIDDEN:2 * HIDDEN],
                         start=False, stop=True)

        ob = smpool.tile([128, 1], f32, tag="ob")
        for k in range(KT):
            nc.tensor.matmul(ob[:], stateT_sb[:, k, bsl],
                             wsm_sb[:, k, 2 * HIDDEN:2 * HIDDEN + 1],
                             start=(k == 0), stop=False)
        nc.tensor.matmul(ob[:], ones_sb[:, 0:128], bias_sb[:, 2 * HIDDEN:2 * HIDDEN + 1],
                         start=False, stop=True)

        hpre = hpool.tile([128, HIDDEN], f32, tag="hpre")
        nc.vector.tensor_reduce(
            hpre[:], partials[bt][:].rearrange("p g h -> p h g"), axis=AX.X, op=ALU.add,
        )
        hp = hpool.tile([128, HIDDEN], f32, tag="hp")
        nc.vector.tensor_add(hp[:], hpre[:], hs[:])
        # elu(x) = max(x,0) + exp(min(x,0)) - 1
        t0 = hpool.tile([128, HIDDEN], f32, tag="t0")
        nc.vector.tensor_scalar_min(t0[:], hp[:], 0.0)
        ex = hpool.tile([128, HIDDEN], f32, tag="ex")
        nc.scalar.activation(ex[:], t0[:], ACTF.Exp)
        t1 = hpool.tile([128, HIDDEN], f32, tag="t1")
        nc.vector.tensor_scalar_max(t1[:], hp[:], 0.0)
        h2 = hpool.tile([128, HIDDEN], f32, tag="h2")
        nc.vector.tensor_add(h2[:], t1[:], ex[:])
        h3 = hpool.tile([128, HIDDEN], f32, tag="h3")
        nc.vector.tensor_scalar_add(h3[:], h2[:], -1.0)

        scr = hpool.tile([128, HIDDEN], f32, tag="scr")
        nc.vector.tensor_mul(scr[:], h3[:], ow[:])
        qts = hpool.tile([128, 1], f32, tag="qts")
        nc.vector.tensor_reduce(qts[:], scr[:], axis=AX.X, op=ALU.add)
        qt = hpool.tile([128, 1], f32, tag="qt")
        nc.vector.tensor_add(qt[:], qts[:], ob[:, 0:1])
        nc.gpsimd.dma_start(out[bsl, :], qt[:])


def _emit_body_v2(nc, tc, ctx, tile, mybir, dram, pools):
    """[h, b]-oriented pipeline: W stationary, movers = stateT * q[b, n] built
    on DVE (bf16 SBUF 2x); agent-sum accumulates in PSUM; final dot = ones
    matmul over h partitions. ACT nearly idle; PE-bound."""
    import concourse.bass as bass
    bf16 = mybir.dt.bfloat16
    f32 = mybir.dt.float32
    fp8 = mybir.dt.float8e4
    AX = mybir.AxisListType
    ALU = mybir.AluOpType
    ACTF = mybir.ActivationFunctionType
    H2 = HIDDEN // 128              # h-tiles (2)
    KF8 = KT - CFG["fp8k"]          # k-tiles >= KF8 run fp8 DoubleRow

    stateT, q, qT, w_perm, w_small, bmat, bias_small, w8, out = dram
    consts, wpool, mvpool, hpool, pspool = (
        pools["consts"], pools["w"], pools["mv"], pools["h"], pools["ps"])

    dma = (nc.sync if CFG["sync_w_dma"] else nc.gpsimd).dma_start
    qdma = (nc.sync if CFG["qrep_sync"] else nc.gpsimd).dma_start

    # ---- constants; stateT split per k-tile so the first slice lands fast
    stateT_sb = consts.tile([128, KT, BS], bf16, tag="stateT")
    stateT_r = stateT.rearrange("(k p) b -> p k b", p=128)
    for k in range(KT):
        dma(stateT_sb[:, k, :], stateT_r[:, k, :])
    wsm_sb = consts.tile([128, KT, HIDDEN * 2 + 2], bf16, tag="wsm")
    nc.gpsimd.dma_start(wsm_sb[:], w_small.rearrange("(k p) c -> p k c", p=128))
    qT_sb = consts.tile([N_AGENTS, BS], bf16, tag="qT")
    nc.gpsimd.dma_start(qT_sb[:], qT[:, :])
    bmat_sb = consts.tile([N_AGENTS, HIDDEN], bf16, tag="bmat")
    nc.gpsimd.dma_start(bmat_sb[:], bmat[:, :])
    bias_sb = consts.tile([1, HIDDEN * 2 + 2], bf16, tag="bias")
    nc.gpsimd.dma_start(bias_sb[:], bias_small[:, :])
    ones_row = consts.tile([1, BS], bf16, tag="ones_row")
    nc.vector.memset(ones_row[:], 1.0)
    ones_col = consts.tile([128, 1], bf16, tag="ones_col")
    nc.vector.memset(ones_col[:], 1.0)

    NQG = 8
    NQTILES = N_AGENTS // NQG
    qrep = [consts.tile([128, NQG, BS], bf16, tag=f"qrep{g}", name=f"qrep{g}")
            for g in range(NQTILES)]

    Y = [pspool.tile([128, BS], f32, tag=f"Y{ht}", name=f"Y{ht}") for ht in range(H2)]
    OW = [pspool.tile([128, BS], f32, tag=f"OW{ht}", name=f"OW{ht}") for ht in range(H2)]
    QP = pspool.tile([1, BS], f32, tag="QP")

    # ---- small matmuls first: they run during the DMA ramp-up while the
    # first W chunks stream in. Y accumulation group OPENS here (start=True)
    # and is closed by the last big-loop matmul.
    for ht in range(H2):
        hsl = slice(ht * 128, (ht + 1) * 128)
        for k in range(KT):
            nc.tensor.matmul(Y[ht][:], wsm_sb[:, k, hsl], stateT_sb[:, k, :],
                             start=(k == 0), stop=False, skip_group_check=True)
        nc.tensor.matmul(Y[ht][:], bmat_sb[:, hsl], qT_sb[:, :],
                         start=False, stop=False, skip_group_check=True)
        nc.tensor.matmul(Y[ht][:], bias_sb[:, hsl], ones_row[:, :],
                         start=False, stop=False, skip_group_check=True)
        for k in range(KT):
            nc.tensor.matmul(OW[ht][:], wsm_sb[:, k, HIDDEN + ht * 128:HIDDEN + (ht + 1) * 128],
                             stateT_sb[:, k, :],
                             start=(k == 0), stop=False, skip_group_check=True)
        nc.tensor.matmul(OW[ht][:], bias_sb[:, HIDDEN + ht * 128:HIDDEN + (ht + 1) * 128],
                         ones_row[:, :], start=False, stop=True, skip_group_check=True)
    for k in range(KT):
        nc.tensor.matmul(QP[:], wsm_sb[:, k, 2 * HIDDEN:2 * HIDDEN + 1],
                         stateT_sb[:, k, :], start=(k == 0), stop=False,
                         skip_group_check=True)
    nc.tensor.matmul(QP[:], bias_sb[:, 2 * HIDDEN:2 * HIDDEN + 1], ones_row[:, :],
                     start=False, stop=False, skip_group_check=True)
    # q_tot -= sum_h out_w[b, h]  (compensates using elu+1 in the tail)
    for k in range(KT):
        nc.tensor.matmul(QP[:], wsm_sb[:, k, 2 * HIDDEN + 1:2 * HIDDEN + 2],
                         stateT_sb[:, k, :], start=False, stop=False,
                         skip_group_check=True)
    nc.tensor.matmul(QP[:], bias_sb[:, 2 * HIDDEN + 1:2 * HIDDEN + 2], ones_row[:, :],
                     start=False, stop=False, skip_group_check=True)

    # ---- fp8 k-tiles: state slices pre-scaled by 1/s (W carries s on host)
    st8 = []
    if CFG["fp8k"] and CFG["mv8_via"] == "dve":
        for kk in range(CFG["fp8k"]):
            t = hpool.tile([128, BS], bf16, tag=f"st8_{kk}", name=f"st8_{kk}")
            nc.vector.tensor_scalar_mul(t[:], stateT_sb[:, KF8 + kk, :],
                                        1.0 / CFG["fp8_s"])
            st8.append(t)

    # ---- big streamed contraction; qrep slices interleave with the W stream
    w_perm_r = w_perm.rearrange("(k p) n -> p k n", p=128)
    w8_r = w8.rearrange("p (f c) -> p f c", f=NF) if w8 is not None else None
    for f in range(NF):
        if f % (NF // NQTILES) == 0:
            g = f // (NF // NQTILES)
            qsrc = bass.AP(
                tensor=qT.tensor, offset=g * NQG * BS,
                ap=[[0, 128], [BS, NQG], [1, BS]],
            )
            qdma(qrep[g][:], qsrc)
        if KF8 > 0:
            wt = wpool.tile([128, KF8, FCHUNK], bf16, tag="w")
            dma(wt[:], w_perm_r[:, 0:KF8, f * FCHUNK:(f + 1) * FCHUNK])
        if CFG["fp8k"]:
            w8t = wpool.tile([128, CFG["fp8k"], NPC, HIDDEN], fp8, tag="w8")
            w8dma = nc.gpsimd.dma_start if CFG["w8_gp"] else dma
            w8dma(w8t[:].rearrange("p kk j h -> p (kk j h)"), w8_r[:, f, :])
        n0 = NPC * f
        korder = [k for k in CFG["korder"] if k < KT]
        for k in korder:
            last = (f == NF - 1 and k == korder[-1])
            qr = qrep[n0 // NQG][:, n0 % NQG:n0 % NQG + NPC, :]
            if k >= KF8:
                kk = k - KF8
                mv8 = mvpool.tile([128, NPC, BS], fp8, tag="mv8")
                if CFG["mv8_via"] == "act":
                    mvb = mvpool.tile([128, NPC, BS], bf16, tag="mv")
                    st_rep = bass.AP(
                        tensor=stateT_sb.tensor,
                        offset=stateT_sb[:, k, :].offset,
                        ap=[stateT_sb[:].ap[0], [0, NPC], [1, BS]],
                    )
                    nc.vector.tensor_mul(mvb[:], st_rep, qr)
                    nc.scalar.activation(
                        mv8[:].rearrange("p j b -> p (j b)"),
                        mvb[:].rearrange("p j b -> p (j b)"),
                        ACTF.Copy, scale=1.0 / CFG["fp8_s"],
                    )
                else:
                    st_rep = bass.AP(
                        tensor=st8[kk].tensor, offset=st8[kk][:].offset,
                        ap=[st8[kk][:].ap[0], [0, NPC], [1, BS]],
                    )
                    nc.vector.tensor_mul(mv8[:], st_rep, qr)
                for ht in range(H2):
                    if CFG["swil"]:
                        lhsT = w8t[:, kk, ht, :]
                        pm = mybir.MatmulPerfMode.DoubleRowSwInterleave
                    else:
                        lhsT = w8t[:, kk, :, ht * 128:(ht + 1) * 128]
                        pm = mybir.MatmulPerfMode.DoubleRow
                    nc.tensor.matmul(
                        Y[ht][:], lhsT,
                        mv8[:], start=False, stop=last and ht == H2 - 1,
                        perf_mode=pm, skip_group_check=True,
                    )
                continue
            mv2 = mvpool.tile([128, NPC, BS], bf16, tag="mv")
            st_rep = bass.AP(
                tensor=stateT_sb.tensor, offset=stateT_sb[:, k, :].offset,
                ap=[stateT_sb[:].ap[0], [0, NPC], [1, BS]],
            )
            nc.vector.tensor_mul(mv2[:], st_rep, qr)
            for j in range(NPC):
                for ht in range(H2):
                    nc.tensor.matmul(
                        Y[ht][:], wt[:, k, j * HIDDEN + ht * 128:j * HIDDEN + (ht + 1) * 128],
                        mv2[:, j, :], start=False,
                        stop=last and j == NPC - 1, skip_group_check=True,
                    )

    # ---- tail: elu + dot with out_w^T, h-reduction via ones matmul.
    # Split along b so the chain pipelines in smaller quanta.
    NBH = 1
    BH = BS // NBH
    for ht in range(H2):
        for hb in range(NBH):
            bsl = slice(hb * BH, (hb + 1) * BH)
            t0 = hpool.tile([128, BH], f32, tag="t0")
            nc.vector.tensor_scalar_min(t0[:], Y[ht][:, bsl], 0.0)
            ex = hpool.tile([128, BH], f32, tag="ex")
            nc.scalar.activation(ex[:], t0[:], ACTF.Exp)
            rl = hpool.tile([128, BH], f32, tag="rl")
            nc.scalar.activation(rl[:], Y[ht][:, bsl], ACTF.Relu)
            h3 = hpool.tile([128, BH], f32, tag="h3")
            nc.vector.tensor_add(h3[:], rl[:], ex[:])
            mT = hpool.tile([128, BH], bf16, tag="mT")
            nc.vector.tensor_mul(mT[:], h3[:], OW[ht][:, bsl])
            nc.tensor.matmul(QP[:, bsl], ones_col[:, 0:1], mT[:],
                             start=False, stop=(ht == H2 - 1 and hb == NBH - 1),
                             skip_group_check=True)
    qrow = hpool.tile([1, BS], f32, tag="qrow")
    nc.scalar.activation(qrow[:], QP[:], ACTF.Copy)
    nc.gpsimd.dma_start(out[:, :], qrow[:])


def build_module(reps=1, loop_reps=1):
    """Build and compile the per-core Bass module. reps>1 repeats the whole
    computation in one NEFF (for timing)."""
    from contextlib import ExitStack
    import concourse.bass as bass
    import concourse.tile as tile
    from concourse import bacc, mybir

    bf16 = mybir.dt.bfloat16
    f32 = mybir.dt.float32

    nc = bacc.Bacc("TRN2", target_bir_lowering=False)
    stateT = nc.dram_tensor("stateT", [STATE_DIM, BS], bf16, kind="ExternalInput").ap()
    q = nc.dram_tensor("q", [BS, N_AGENTS], f32, kind="ExternalInput").ap()
    qT = nc.dram_tensor("qT", [N_AGENTS, BS], bf16, kind="ExternalInput").ap()
    w_perm = nc.dram_tensor("w_perm", [STATE_DIM, HIDDEN * N_AGENTS], bf16,
                            kind="ExternalInput").ap()
    w_small = nc.dram_tensor("w_small", [STATE_DIM, HIDDEN * 2 + 2], bf16,
                             kind="ExternalInput").ap()
    bmat = nc.dram_tensor("bmat", [N_AGENTS, HIDDEN], bf16, kind="ExternalInput").ap()
    bias_small = nc.dram_tensor("bias_small", [1, HIDDEN * 2 + 2], bf16,
                                kind="ExternalInput").ap()
    w8 = None
    if CFG["arch"] == "v2" and CFG["fp8k"]:
        w8 = nc.dram_tensor(
            "w8", [128, NF * CFG["fp8k"] * NPC * HIDDEN], mybir.dt.float8e4,
            kind="ExternalInput").ap()
    if CFG["arch"] == "v2":
        out = nc.dram_tensor("out", [1, BS], f32, kind="ExternalOutput").ap()
        dram = (stateT, q, qT, w_perm, w_small, bmat, bias_small, w8, out)
    else:
        out = nc.dram_tensor("out", [BS, 1], f32, kind="ExternalOutput").ap()
        dram = (stateT, q, qT, w_perm, w_small, bmat, bias_small, out)

    with tile.TileContext(nc) as tc:
        with ExitStack() as ctx:
            if CFG["arch"] == "v2":
                pools = {
                    "bass": bass,
                    "consts": ctx.enter_context(tc.tile_pool(name="consts", bufs=1)),
                    "w": ctx.enter_context(tc.tile_pool(name="w", bufs=CFG["w_bufs"])),
                    "mv": ctx.enter_context(tc.tile_pool(name="mv", bufs=CFG["mv_bufs"])),
                    "h": ctx.enter_context(tc.tile_pool(name="h", bufs=2)),
                    "ps": ctx.enter_context(tc.tile_pool(name="ps", bufs=1, space="PSUM")),
                }
                emit = _emit_body_v2
            else:
                pools = {
                    "bass": bass,
                    "consts": ctx.enter_context(tc.tile_pool(name="consts", bufs=1)),
                    "w": ctx.enter_context(tc.tile_pool(name="w", bufs=CFG["w_bufs"])),
                    "s": ctx.enter_context(tc.tile_pool(name="s", bufs=2)),
                    "pa": ctx.enter_context(tc.tile_pool(name="pa", bufs=1)),
                    "h": ctx.enter_context(tc.tile_pool(name="h", bufs=2)),
                    "ps": ctx.enter_context(tc.tile_pool(name="ps", bufs=CFG["ps_bufs"], space="PSUM")),
                    "sm": ctx.enter_context(tc.tile_pool(name="sm", bufs=1, space="PSUM")),
                }
                emit = _emit_body
            if loop_reps > 1:
                with tc.For_i(0, loop_reps, 1,
                              hint_engines=(mybir.EngineType.PE,
                                            mybir.EngineType.DVE)):
                    for _ in range(reps):
                        emit(nc, tc, ctx, tile, mybir, dram, pools)
            else:
                for _ in range(reps):
                    emit(nc, tc, ctx, tile, mybir, dram, pools)
    nc.compile()
    return nc


def make_in_maps(q_values, state_representation, W_hw, b_hw, W_ow, b_ow, W_hb,
                 b_hb, W_ob, b_ob):
    bf16 = ml_dtypes.bfloat16
    q = np.asarray(q_values, dtype=np.float32).reshape(B, N_AGENTS)
    st = np.asarray(state_representation, dtype=np.float32)
    W_hw = np.asarray(W_hw, dtype=np.float32)
    # permute columns of W_hw from (h, n) to (n, h) order
    w_perm = np.ascontiguousarray(
        W_hw.reshape(STATE_DIM, HIDDEN, N_AGENTS).transpose(0, 2, 1)
        .reshape(STATE_DIM, HIDDEN * N_AGENTS)).astype(bf16)
    W_ow32 = np.asarray(W_ow, np.float32)
    w_small = np.ascontiguousarray(np.concatenate(
        [np.asarray(W_hb, np.float32), W_ow32,
         np.asarray(W_ob, np.float32),
         -W_ow32.sum(axis=1, keepdims=True)], axis=1)).astype(bf16)
    bmat = np.ascontiguousarray(
        np.asarray(b_hw, np.float32).reshape(HIDDEN, N_AGENTS).T).astype(bf16)
    b_ow32 = np.asarray(b_ow, np.float32)
    bias_small = np.concatenate(
        [np.asarray(b_hb, np.float32), b_ow32,
         np.asarray(b_ob, np.float32),
         -b_ow32.sum(keepdims=True)]).reshape(1, HIDDEN * 2 + 2).astype(bf16)
    w8a = None
    if CFG["arch"] == "v2" and CFG["fp8k"]:
        fp8k, kf8 = CFG["fp8k"], KT - CFG["fp8k"]
        wp3 = W_hw.reshape(STATE_DIM, HIDDEN, N_AGENTS).transpose(0, 2, 1)
        # [kk, p, f, j, h] -> [p, (f, kk, j, h)]
        arr = wp3[kf8 * 128:].reshape(fp8k, 128, NF, NPC, HIDDEN)
        if CFG["swil"]:
            # flat col = 2*(127 - h_local) + j, per (kk, ht): the HW reads
            # SW-interleaved weights contiguously (pairs adjacent, reversed)
            a6 = arr.reshape(fp8k, 128, NF, NPC, HIDDEN // 128, 128)
            a6 = a6.transpose(1, 2, 0, 4, 5, 3)[:, :, :, :, ::-1, :]
            w8a = np.ascontiguousarray(
                a6 * CFG["fp8_s"]
            ).astype(ml_dtypes.float8_e4m3).reshape(128, -1)
        else:
            w8a = np.ascontiguousarray(
                arr.transpose(1, 2, 0, 3, 4) * CFG["fp8_s"]
            ).astype(ml_dtypes.float8_e4m3).reshape(128, -1)
    in_maps = []
    for c in range(N_CORES):
        sl = slice(c * BS, (c + 1) * BS)
        m = {
            "stateT": np.ascontiguousarray(st[sl].T).astype(bf16),
            "q": np.ascontiguousarray(q[sl]),
            "qT": np.ascontiguousarray(q[sl].T).astype(bf16),
            "w_perm": w_perm,
            "w_small": w_small,
            "bmat": bmat,
            "bias_small": bias_small,
        }
        if w8a is not None:
            m["w8"] = w8a
        in_maps.append(m)
    return in_maps


def kernel(**inputs):
    from concourse.bass_utils import run_bass_kernel_spmd

    if "nc" not in _CACHE:
        _CACHE["nc"] = build_module()
    nc = _CACHE["nc"]
    in_maps = make_in_maps(**inputs)
    res = run_bass_kernel_spmd(nc, in_maps, core_ids=list(range(N_CORES)))
    if CFG["arch"] == "v2":
        out = np.concatenate(
            [res.results[c]["out"][0] for c in range(N_CORES)]).reshape(B, 1)
    else:
        out = np.concatenate([res.results[c]["out"] for c in range(N_CORES)], axis=0)
    return out.astype(np.float32)



# revision 22
# speedup vs baseline: 210.8220x; 1.1990x over previous
"""Trainium2 Bass kernel for nn_MixingNetwork (QMIX-style mixer).

Math (per sample b):
  hid_w = (state @ W_hw).reshape(H, N); out_w = state @ W_ow; hid_b = state @ W_hb
  h     = elu(hid_w @ q + hid_b);      q_tot = out_w . h + state @ W_ob (+ biases)

Strategy: pure data parallel over batch (512 samples/core on 8 cores).
v2 arch ([h, b]-oriented): W stationary on the PE; movers = stateT * q[b, n]
built on DVE in bf16 (DVE tensor_tensor runs at 1 elem/cycle -- ~80us, the
second wall after the PE's ~98us); agent sums accumulate in PSUM.

fp8 hybrid: k-tile 3 (1/4 of the contraction) runs as fp8e4m3 DoubleRow,
pairing the 2 agents of an f-chunk into one matmul (2x PE rate). More fp8
fails the 2e-2 gate: full-fp8 measures 3.4e-2, 1/4 measures 1.73e-2
(e4m3's 3-bit mantissa on both operands; one-side-exact splits cost the
pairing back). The fp8 movers are cast bf16->fp8 on the otherwise-idle ACT
engine (Copy, scale=1/4; direct fp8-out TT on DVE is 2x slower and makes
DVE the critical path). W8 carries the balancing 4x on the host.
All biases are folded in as rank-1 / small matmuls accumulated in PSUM;
the elu+1 compensation column is host-folded into W_ob (QP: one pass).
"""

import numpy as np
import ml_dtypes

B, N_AGENTS, HIDDEN, STATE_DIM = 4096, 64, 256, 512
N_CORES = 8
BS = B // N_CORES          # samples per core
NBT = BS // 128            # b-tiles per core
KT = STATE_DIM // 128      # k-tiles over state dim
FCHUNK = 512               # columns of W per PSUM chunk
NF = HIDDEN * N_AGENTS // FCHUNK   # 32 chunks
NPC = FCHUNK // HIDDEN     # agents (n) per chunk = 2
GROUP = 8                  # f-chunks per partial-reduce group
NG = NF // GROUP           # groups

_CACHE = {}

# build-time tuning knobs (A/B testing)
CFG = {
    "dve_every": 4,      # every Nth scale op on DVE (0 = all on ACT)
    "sync_w_dma": True,  # W-chunk DMAs via HWDGE (sync engine)
    "ps_bufs": 4,        # big-psum pool bufs
    "w_bufs": 6,
    "mode": "full",     # "dma" | "mm" | "full" — partial builds for HW bisect
    "arch": "v2",       # v1: scale-from-PSUM + reduce; v2: pre-scaled movers
    "mv_bufs": 24,
    "qrep_sync": True,  # qrep/const broadcast DMAs via HWDGE
    "mv_pair": True,    # one DVE op builds movers for both agents of a chunk
    "fp8k": 1,          # of the 4 k-tiles, how many (from the top) run as
                        # fp8e4m3 DoubleRow (2 agents/MM). err ~1.7e-2 at 1.
    "fp8_s": 4.0,       # balance scale: W*s on host, state/s on device
    "mv8_via": "act",   # "act": DVE builds bf16 mover, ACT casts to fp8
                        # (keeps DVE at its bf16 rate); "dve": direct fp8 TT
    "korder": (0, 3, 1, 2),  # emission order of k-tiles within an f-chunk:
                        # DR matmuls mid-chunk hide their longer LDWEIGHTS
    "w8_gp": False,     # w8t DMAs on the Pool queue (measured: neutral)
    "swil": False,      # DoubleRowSwInterleave: host pre-interleaves W pairs
                        # (contiguous LDWEIGHTS, no HW reversal penalty)
}


def _emit_body(nc, tc, ctx, tile, mybir, dram, pools):
    bass = pools["bass"]
    bf16 = mybir.dt.bfloat16
    f32 = mybir.dt.float32
    AX = mybir.AxisListType
    ALU = mybir.AluOpType
    ACTF = mybir.ActivationFunctionType

    stateT, q, qT, w_perm, w_small, bmat, bias_small, out = dram
    consts, wpool, spool, papool, hpool, pspool, smpool = (
        pools["consts"], pools["w"], pools["s"], pools["pa"], pools["h"],
        pools["ps"], pools["sm"],
    )

    # ---- constants into SBUF (emitted per rep; cheap) ----
    stateT_sb = consts.tile([128, KT, BS], bf16, tag="stateT")
    nc.sync.dma_start(stateT_sb[:], stateT.rearrange("(k p) b -> p k b", p=128))
    q_sb = consts.tile([128, NBT, N_AGENTS], f32, tag="q")
    nc.gpsimd.dma_start(q_sb[:], q.rearrange("(t p) n -> p t n", p=128))
    wsm_sb = consts.tile([128, KT, HIDDEN * 2 + 2], bf16, tag="wsm")
    nc.gpsimd.dma_start(wsm_sb[:], w_small.rearrange("(k p) c -> p k c", p=128))
    qT_sb = consts.tile([N_AGENTS, BS], bf16, tag="qT")
    nc.gpsimd.dma_start(qT_sb[:], qT[:, :])
    bmat_sb = consts.tile([N_AGENTS, HIDDEN], bf16, tag="bmat")
    nc.gpsimd.dma_start(bmat_sb[:], bmat[:, :])
    bias_sb = consts.tile([1, HIDDEN * 2 + 2], bf16, tag="bias")
    nc.gpsimd.dma_start(bias_sb[:], bias_small[:, :])
    ones_sb = consts.tile([1, 128], bf16, tag="ones")
    nc.vector.memset(ones_sb[:], 1.0)

    partials = [papool.tile([128, NG, HIDDEN], f32, tag=f"pa{bt}", name=f"pa{bt}")
                for bt in range(NBT)]
    S = [None] * NBT

    # ---- big contraction: G = stateT.T @ W_perm, scaled by q, reduced over n ----
    # fraction of scale ops routed to DVE (rest on ACT) to balance engines
    ndve = 0
    for f in range(NF):
        wt = wpool.tile([128, KT, FCHUNK], bf16, tag="w")
        dma_eng = nc.sync if CFG["sync_w_dma"] else nc.gpsimd
        dma_eng.dma_start(
            wt[:],
            w_perm.rearrange("(k p) n -> p k n", p=128)[:, :, f * FCHUNK:(f + 1) * FCHUNK],
        )
        g, pos = divmod(f, GROUP)
        for bt in range(NBT):
            if pos == 0:
                S[bt] = spool.tile([128, NPC * GROUP, HIDDEN], bf16, tag=f"S{bt}",
                                   name=f"S{bt}")
            if CFG["mode"] == "dma":
                continue
            ps = pspool.tile([128, FCHUNK], f32, tag="ps")
            bsl = slice(bt * 128, (bt + 1) * 128)
            for k in range(KT):
                nc.tensor.matmul(
                    ps[:], stateT_sb[:, k, bsl], wt[:, k, :],
                    start=(k == 0), stop=(k == KT - 1),
                )
            for j in range(NPC if CFG["mode"] == "full" else 0):
                n_local = NPC * pos + j
                n_glob = NPC * f + j
                ndve += 1
                if CFG["dve_every"] and ndve % CFG["dve_every"] == 0:
                    nc.vector.tensor_scalar_mul(
                        S[bt][:, n_local, :], ps[:, j * HIDDEN:(j + 1) * HIDDEN],
                        q_sb[:, bt, n_glob:n_glob + 1],
                    )
                else:
                    nc.scalar.activation(
                        S[bt][:, n_local, :], ps[:, j * HIDDEN:(j + 1) * HIDDEN],
                        ACTF.Copy, scale=q_sb[:, bt, n_glob:n_glob + 1],
                    )
            if pos == GROUP - 1 and CFG["mode"] == "full":
                nc.vector.tensor_reduce(
                    partials[bt][:, g, :],
                    S[bt][:].rearrange("p n h -> p h n"),
                    axis=AX.X, op=ALU.add,
                )

    # ---- per-b-tile tail: hypernet biases + small matmuls + ELU + final dot ----
    for bt in range(NBT if CFG["mode"] == "full" else 0):
        bsl = slice(bt * 128, (bt + 1) * 128)
        hs = smpool.tile([128, HIDDEN], f32, tag="hsum")
        for k in range(KT):
            nc.tensor.matmul(hs[:], stateT_sb[:, k, bsl], wsm_sb[:, k, 0:HIDDEN],
                             start=(k == 0), stop=False)
        nc.tensor.matmul(hs[:], qT_sb[:, bsl], bmat_sb[:], start=False, stop=False)
        nc.tensor.matmul(hs[:], ones_sb[:, 0:128], bias_sb[:, 0:HIDDEN],
                         start=False, stop=True)

        ow = smpool.tile([128, HIDDEN], f32, tag="ow")
        for k in range(KT):
            nc.tensor.matmul(ow[:], stateT_sb[:, k, bsl],
                             wsm_sb[:, k, HIDDEN:2 * HIDDEN],
                             start=(k == 0), stop=False)
        nc.tensor.matmul(ow[:], ones_sb[:, 0:128], bias_sb[:, HIDDEN:2 * HIDDEN],
                         start=False, stop=True)

        ob = smpool.tile([128, 1], f32, tag="ob")
        for k in range(KT):
            nc.tensor.matmul(ob[:], stateT_sb[:, k, bsl],
                             wsm_sb[:, k, 2 * HIDDEN:2 * HIDDEN + 1],
                             start=(k == 0), stop=False)
        nc.tensor.matmul(ob[:], ones_sb[:, 0:128], bias_sb[:, 2 * HIDDEN:2 * HIDDEN + 1],
                         start=False, stop=True)

        hpre = hpool.tile([128, HIDDEN], f32, tag="hpre")
        nc.vector.tensor_reduce(
            hpre[:], partials[bt][:].rearrange("p g h -> p h g"), axis=AX.X, op=ALU.add,
        )
        hp = hpool.tile([128, HIDDEN], f32, tag="hp")
        nc.vector.tensor_add(hp[:], hpre[:], hs[:])
        # elu(x) = max(x,0) + exp(min(x,0)) - 1
        t0 = hpool.tile([128, HIDDEN], f32, tag="t0")
        nc.vector.tensor_scalar_min(t0[:], hp[:], 0.0)
        ex = hpool.tile([128, HIDDEN], f32, tag="ex")
        nc.scalar.activation(ex[:], t0[:], ACTF.Exp)
        t1 = hpool.tile([128, HIDDEN], f32, tag="t1")
        nc.vector.tensor_scalar_max(t1[:], hp[:], 0.0)
        h2 = hpool.tile([128, HIDDEN], f32, tag="h2")
        nc.vector.tensor_add(h2[:], t1[:], ex[:])
        h3 = hpool.tile([128, HIDDEN], f32, tag="h3")
        nc.vector.tensor_scalar_add(h3[:], h2[:], -1.0)

        scr = hpool.tile([128, HIDDEN], f32, tag="scr")
        nc.vector.tensor_mul(scr[:], h3[:], ow[:])
        qts = hpool.tile([128, 1], f32, tag="qts")
        nc.vector.tensor_reduce(qts[:], scr[:], axis=AX.X, op=ALU.add)
        qt = hpool.tile([128, 1], f32, tag="qt")
        nc.vector.tensor_add(qt[:], qts[:], ob[:, 0:1])
        nc.gpsimd.dma_start(out[bsl, :], qt[:])


def _emit_body_v2(nc, tc, ctx, tile, mybir, dram, pools):
    """[h, b]-oriented pipeline: W stationary, movers = stateT * q[b, n] built
    on DVE (bf16 SBUF 2x); agent-sum accumulates in PSUM; final dot = ones
    matmul over h partitions. ACT nearly idle; PE-bound."""
    import concourse.bass as bass
    bf16 = mybir.dt.bfloat16
    f32 = mybir.dt.float32
    fp8 = mybir.dt.float8e4
    AX = mybir.AxisListType
    ALU = mybir.AluOpType
    ACTF = mybir.ActivationFunctionType
    H2 = HIDDEN // 128              # h-tiles (2)
    KF8 = KT - CFG["fp8k"]          # k-tiles >= KF8 run fp8 DoubleRow

    stateT, q, qT, w_perm, w_small, bmat, bias_small, w8, out = dram
    consts, wpool, mvpool, hpool, pspool = (
        pools["consts"], pools["w"], pools["mv"], pools["h"], pools["ps"])

    dma = (nc.sync if CFG["sync_w_dma"] else nc.gpsimd).dma_start
    qdma = (nc.sync if CFG["qrep_sync"] else nc.gpsimd).dma_start

    # ---- constants; stateT split per k-tile so the first slice lands fast
    stateT_sb = consts.tile([128, KT, BS], bf16, tag="stateT")
    stateT_r = stateT.rearrange("(k p) b -> p k b", p=128)
    for k in range(KT):
        dma(stateT_sb[:, k, :], stateT_r[:, k, :])
    wsm_sb = consts.tile([128, KT, HIDDEN * 2 + 2], bf16, tag="wsm")
    nc.gpsimd.dma_start(wsm_sb[:], w_small.rearrange("(k p) c -> p k c", p=128))
    qT_sb = consts.tile([N_AGENTS, BS], bf16, tag="qT")
    nc.gpsimd.dma_start(qT_sb[:], qT[:, :])
    bmat_sb = consts.tile([N_AGENTS, HIDDEN], bf16, tag="bmat")
    nc.gpsimd.dma_start(bmat_sb[:], bmat[:, :])
    bias_sb = consts.tile([1, HIDDEN * 2 + 2], bf16, tag="bias")
    nc.gpsimd.dma_start(bias_sb[:], bias_small[:, :])
    ones_row = consts.tile([1, BS], bf16, tag="ones_row")
    nc.vector.memset(ones_row[:], 1.0)
    ones_col = consts.tile([128, 1], bf16, tag="ones_col")
    nc.vector.memset(ones_col[:], 1.0)

    NQG = 8
    NQTILES = N_AGENTS // NQG
    qrep = [consts.tile([128, NQG, BS], bf16, tag=f"qrep{g}", name=f"qrep{g}")
            for g in range(NQTILES)]

    Y = [pspool.tile([128, BS], f32, tag=f"Y{ht}", name=f"Y{ht}") for ht in range(H2)]
    OW = [pspool.tile([128, BS], f32, tag=f"OW{ht}", name=f"OW{ht}") for ht in range(H2)]
    QP = pspool.tile([1, BS], f32, tag="QP")

    # ---- small matmuls first: they run during the DMA ramp-up while the
    # first W chunks stream in. Y accumulation group OPENS here (start=True)
    # and is closed by the last big-loop matmul.
    for ht in range(H2):
        hsl = slice(ht * 128, (ht + 1) * 128)
        for k in range(KT):
            nc.tensor.matmul(Y[ht][:], wsm_sb[:, k, hsl], stateT_sb[:, k, :],
                             start=(k == 0), stop=False, skip_group_check=True)
        nc.tensor.matmul(Y[ht][:], bmat_sb[:, hsl], qT_sb[:, :],
                         start=False, stop=False, skip_group_check=True)
        nc.tensor.matmul(Y[ht][:], bias_sb[:, hsl], ones_row[:, :],
                         start=False, stop=False, skip_group_check=True)
        for k in range(KT):
            nc.tensor.matmul(OW[ht][:], wsm_sb[:, k, HIDDEN + ht * 128:HIDDEN + (ht + 1) * 128],
                             stateT_sb[:, k, :],
                             start=(k == 0), stop=False, skip_group_check=True)
        nc.tensor.matmul(OW[ht][:], bias_sb[:, HIDDEN + ht * 128:HIDDEN + (ht + 1) * 128],
                         ones_row[:, :], start=False, stop=True, skip_group_check=True)
    # w_small col 2H already holds W_ob - sum_h W_ow (host-folded); the
    # -sum_h out_w term compensates using elu+1 in the tail
    for k in range(KT):
        nc.tensor.matmul(QP[:], wsm_sb[:, k, 2 * HIDDEN:2 * HIDDEN + 1],
                         stateT_sb[:, k, :], start=(k == 0), stop=False,
                         skip_group_check=True)
    nc.tensor.matmul(QP[:], bias_sb[:, 2 * HIDDEN:2 * HIDDEN + 1], ones_row[:, :],
                     start=False, stop=False, skip_group_check=True)

    # ---- fp8 k-tiles: state slices pre-scaled by 1/s (W carries s on host)
    st8 = []
    if CFG["fp8k"] and CFG["mv8_via"] == "dve":
        for kk in range(CFG["fp8k"]):
            t = hpool.tile([128, BS], bf16, tag=f"st8_{kk}", name=f"st8_{kk}")
            nc.vector.tensor_scalar_mul(t[:], stateT_sb[:, KF8 + kk, :],
                                        1.0 / CFG["fp8_s"])
            st8.append(t)

    # ---- big streamed contraction; qrep slices interleave with the W stream
    w_perm_r = w_perm.rearrange("(k p) n -> p k n", p=128)
    w8_r = w8.rearrange("p (f c) -> p f c", f=NF) if w8 is not None else None
    for f in range(NF):
        if f % (NF // NQTILES) == 0:
            g = f // (NF // NQTILES)
            qsrc = bass.AP(
                tensor=qT.tensor, offset=g * NQG * BS,
                ap=[[0, 128], [BS, NQG], [1, BS]],
            )
            qdma(qrep[g][:], qsrc)
        if KF8 > 0:
            wt = wpool.tile([128, KF8, FCHUNK], bf16, tag="w")
            dma(wt[:], w_perm_r[:, 0:KF8, f * FCHUNK:(f + 1) * FCHUNK])
        if CFG["fp8k"]:
            w8t = wpool.tile([128, CFG["fp8k"], NPC, HIDDEN], fp8, tag="w8")
            w8dma = nc.gpsimd.dma_start if CFG["w8_gp"] else dma
            w8dma(w8t[:].rearrange("p kk j h -> p (kk j h)"), w8_r[:, f, :])
        n0 = NPC * f
        korder = [k for k in CFG["korder"] if k < KT]
        for k in korder:
            last = (f == NF - 1 and k == korder[-1])
            qr = qrep[n0 // NQG][:, n0 % NQG:n0 % NQG + NPC, :]
            if k >= KF8:
                kk = k - KF8
                mv8 = mvpool.tile([128, NPC, BS], fp8, tag="mv8")
                if CFG["mv8_via"] == "act":
                    mvb = mvpool.tile([128, NPC, BS], bf16, tag="mv")
                    st_rep = bass.AP(
                        tensor=stateT_sb.tensor,
                        offset=stateT_sb[:, k, :].offset,
                        ap=[stateT_sb[:].ap[0], [0, NPC], [1, BS]],
                    )
                    nc.vector.tensor_mul(mvb[:], st_rep, qr)
                    nc.scalar.activation(
                        mv8[:].rearrange("p j b -> p (j b)"),
                        mvb[:].rearrange("p j b -> p (j b)"),
                        ACTF.Copy, scale=1.0 / CFG["fp8_s"],
                    )
                else:
                    st_rep = bass.AP(
                        tensor=st8[kk].tensor, offset=st8[kk][:].offset,
                        ap=[st8[kk][:].ap[0], [0, NPC], [1, BS]],
                    )
                    nc.vector.tensor_mul(mv8[:], st_rep, qr)
                for ht in range(H2):
                    if CFG["swil"]:
                        lhsT = w8t[:, kk, ht, :]
                        pm = mybir.MatmulPerfMode.DoubleRowSwInterleave
                    else:
                        lhsT = w8t[:, kk, :, ht * 128:(ht + 1) * 128]
                        pm = mybir.MatmulPerfMode.DoubleRow
                    nc.tensor.matmul(
                        Y[ht][:], lhsT,
                        mv8[:], start=False, stop=last and ht == H2 - 1,
                        perf_mode=pm, skip_group_check=True,
                    )
                continue
            mv2 = mvpool.tile([128, NPC, BS], bf16, tag="mv")
            st_rep = bass.AP(
                tensor=stateT_sb.tensor, offset=stateT_sb[:, k, :].offset,
                ap=[stateT_sb[:].ap[0], [0, NPC], [1, BS]],
            )
            nc.vector.tensor_mul(mv2[:], st_rep, qr)
            for j in range(NPC):
                for ht in range(H2):
                    nc.tensor.matmul(
                        Y[ht][:], wt[:, k, j * HIDDEN + ht * 128:j * HIDDEN + (ht + 1) * 128],
                        mv2[:, j, :], start=False,
                        stop=last and j == NPC - 1, skip_group_check=True,
                    )

    # ---- tail: elu + dot with out_w^T, h-reduction via ones matmul.
    # Split along b so the chain pipelines in smaller quanta.
    NBH = 1
    BH = BS // NBH
    for ht in range(H2):
        for hb in range(NBH):
            bsl = slice(hb * BH, (hb + 1) * BH)
            t0 = hpool.tile([128, BH], f32, tag="t0")
            nc.vector.tensor_scalar_min(t0[:], Y[ht][:, bsl], 0.0)
            ex = hpool.tile([128, BH], f32, tag="ex")
            nc.scalar.activation(ex[:], t0[:], ACTF.Exp)
            rl = hpool.tile([128, BH], f32, tag="rl")
            nc.scalar.activation(rl[:], Y[ht][:, bsl], ACTF.Relu)
            h3 = hpool.tile([128, BH], f32, tag="h3")
            nc.vector.tensor_add(h3[:], rl[:], ex[:])
            mT = hpool.tile([128, BH], bf16, tag="mT")
            nc.vector.tensor_mul(mT[:], h3[:], OW[ht][:, bsl])
            nc.tensor.matmul(QP[:, bsl], ones_col[:, 0:1], mT[:],
                             start=False, stop=(ht == H2 - 1 and hb == NBH - 1),
                             skip_group_check=True)
    qrow = hpool.tile([1, BS], f32, tag="qrow")
    nc.scalar.activation(qrow[:], QP[:], ACTF.Copy)
    nc.gpsimd.dma_start(out[:, :], qrow[:])


def build_module(reps=1, loop_reps=1):
    """Build and compile the per-core Bass module. reps>1 repeats the whole
    computation in one NEFF (for timing)."""
    from contextlib import ExitStack
    import concourse.bass as bass
    import concourse.tile as tile
    from concourse import bacc, mybir

    bf16 = mybir.dt.bfloat16
    f32 = mybir.dt.float32

    nc = bacc.Bacc("TRN2", target_bir_lowering=False)
    stateT = nc.dram_tensor("stateT", [STATE_DIM, BS], bf16, kind="ExternalInput").ap()
    q = nc.dram_tensor("q", [BS, N_AGENTS], f32, kind="ExternalInput").ap()
    qT = nc.dram_tensor("qT", [N_AGENTS, BS], bf16, kind="ExternalInput").ap()
    w_perm = nc.dram_tensor("w_perm", [STATE_DIM, HIDDEN * N_AGENTS], bf16,
                            kind="ExternalInput").ap()
    w_small = nc.dram_tensor("w_small", [STATE_DIM, HIDDEN * 2 + 2], bf16,
                             kind="ExternalInput").ap()
    bmat = nc.dram_tensor("bmat", [N_AGENTS, HIDDEN], bf16, kind="ExternalInput").ap()
    bias_small = nc.dram_tensor("bias_small", [1, HIDDEN * 2 + 2], bf16,
                                kind="ExternalInput").ap()
    w8 = None
    if CFG["arch"] == "v2" and CFG["fp8k"]:
        w8 = nc.dram_tensor(
            "w8", [128, NF * CFG["fp8k"] * NPC * HIDDEN], mybir.dt.float8e4,
            kind="ExternalInput").ap()
    if CFG["arch"] == "v2":
        out = nc.dram_tensor("out", [1, BS], f32, kind="ExternalOutput").ap()
        dram = (stateT, q, qT, w_perm, w_small, bmat, bias_small, w8, out)
    else:
        out = nc.dram_tensor("out", [BS, 1], f32, kind="ExternalOutput").ap()
        dram = (stateT, q, qT, w_perm, w_small, bmat, bias_small, out)

    with tile.TileContext(nc) as tc:
        with ExitStack() as ctx:
            if CFG["arch"] == "v2":
                pools = {
                    "bass": bass,
                    "consts": ctx.enter_context(tc.tile_pool(name="consts", bufs=1)),
                    "w": ctx.enter_context(tc.tile_pool(name="w", bufs=CFG["w_bufs"])),
                    "mv": ctx.enter_context(tc.tile_pool(name="mv", bufs=CFG["mv_bufs"])),
                    "h": ctx.enter_context(tc.tile_pool(name="h", bufs=2)),
                    "ps": ctx.enter_context(tc.tile_pool(name="ps", bufs=1, space="PSUM")),
                }
                emit = _emit_body_v2
            else:
                pools = {
                    "bass": bass,
                    "consts": ctx.enter_context(tc.tile_pool(name="consts", bufs=1)),
                    "w": ctx.enter_context(tc.tile_pool(name="w", bufs=CFG["w_bufs"])),
                    "s": ctx.enter_context(tc.tile_pool(name="s", bufs=2)),
                    "pa": ctx.enter_context(tc.tile_pool(name="pa", bufs=1)),
                    "h": ctx.enter_context(tc.tile_pool(name="h", bufs=2)),
                    "ps": ctx.enter_context(tc.tile_pool(name="ps", bufs=CFG["ps_bufs"], space="PSUM")),
                    "sm": ctx.enter_context(tc.tile_pool(name="sm", bufs=1, space="PSUM")),
                }
                emit = _emit_body
            if loop_reps > 1:
                with tc.For_i(0, loop_reps, 1,
                              hint_engines=(mybir.EngineType.PE,
                                            mybir.EngineType.DVE)):
                    for _ in range(reps):
                        emit(nc, tc, ctx, tile, mybir, dram, pools)
            else:
                for _ in range(reps):
                    emit(nc, tc, ctx, tile, mybir, dram, pools)
    nc.compile()
    return nc


def make_in_maps(q_values, state_representation, W_hw, b_hw, W_ow, b_ow, W_hb,
                 b_hb, W_ob, b_ob):
    bf16 = ml_dtypes.bfloat16
    q = np.asarray(q_values, dtype=np.float32).reshape(B, N_AGENTS)
    st = np.asarray(state_representation, dtype=np.float32)
    W_hw = np.asarray(W_hw, dtype=np.float32)
    # permute columns of W_hw from (h, n) to (n, h) order
    w_perm = np.ascontiguousarray(
        W_hw.reshape(STATE_DIM, HIDDEN, N_AGENTS).transpose(0, 2, 1)
        .reshape(STATE_DIM, HIDDEN * N_AGENTS)).astype(bf16)
    W_ow32 = np.asarray(W_ow, np.float32)
    ob_col = np.asarray(W_ob, np.float32)
    ob_bias = np.asarray(b_ob, np.float32)
    b_ow32 = np.asarray(b_ow, np.float32)
    if CFG["arch"] == "v2":
        # fold the elu+1 compensation (-sum_h W_ow / -sum_h b_ow) into the
        # W_ob column so QP needs one contraction instead of two
        ob_col = ob_col - W_ow32.sum(axis=1, keepdims=True)
        ob_bias = ob_bias - b_ow32.sum(keepdims=True)
    w_small = np.ascontiguousarray(np.concatenate(
        [np.asarray(W_hb, np.float32), W_ow32, ob_col,
         -W_ow32.sum(axis=1, keepdims=True)], axis=1)).astype(bf16)
    bmat = np.ascontiguousarray(
        np.asarray(b_hw, np.float32).reshape(HIDDEN, N_AGENTS).T).astype(bf16)
    bias_small = np.concatenate(
        [np.asarray(b_hb, np.float32), b_ow32, ob_bias,
         -b_ow32.sum(keepdims=True)]).reshape(1, HIDDEN * 2 + 2).astype(bf16)
    w8a = None
    if CFG["arch"] == "v2" and CFG["fp8k"]:
        fp8k, kf8 = CFG["fp8k"], KT - CFG["fp8k"]
        wp3 = W_hw.reshape(STATE_DIM, HIDDEN, N_AGENTS).transpose(0, 2, 1)
        # [kk, p, f, j, h] -> [p, (f, kk, j, h)]
        arr = wp3[kf8 * 128:].reshape(fp8k, 128, NF, NPC, HIDDEN)
        if CFG["swil"]:
            # flat col = 2*(127 - h_local) + j, per (kk, ht): the HW reads
            # SW-interleaved weights contiguously (pairs adjacent, reversed)
            a6 = arr.reshape(fp8k, 128, NF, NPC, HIDDEN // 128, 128)
            a6 = a6.transpose(1, 2, 0, 4, 5, 3)[:, :, :, :, ::-1, :]
            w8a = np.ascontiguousarray(
                a6 * CFG["fp8_s"]
            ).astype(ml_dtypes.float8_e4m3).reshape(128, -1)
        else:
            w8a = np.ascontiguousarray(
                arr.transpose(1, 2, 0, 3, 4) * CFG["fp8_s"]
            ).astype(ml_dtypes.float8_e4m3).reshape(128, -1)
    in_maps = []
    for c in range(N_CORES):
        sl = slice(c * BS, (c + 1) * BS)
        m = {
            "stateT": np.ascontiguousarray(st[sl].T).astype(bf16),
            "q": np.ascontiguousarray(q[sl]),
            "qT": np.ascontiguousarray(q[sl].T).astype(bf16),
            "w_perm": w_perm,
            "w_small": w_small,
            "bmat": bmat,
            "bias_small": bias_small,
        }
        if w8a is not None:
            m["w8"] = w8a
        in_maps.append(m)
    return in_maps


def kernel(**inputs):
    from concourse.bass_utils import run_bass_kernel_spmd

    if "nc" not in _CACHE:
        _CACHE["nc"] = build_module()
    nc = _CACHE["nc"]
    in_maps = make_in_maps(**inputs)
    res = run_bass_kernel_spmd(nc, in_maps, core_ids=list(range(N_CORES)))
    if CFG["arch"] == "v2":
        out = np.concatenate(
            [res.results[c]["out"][0] for c in range(N_CORES)]).reshape(B, 1)
    else:
        out = np.concatenate([res.results[c]["out"] for c in range(N_CORES)], axis=0)
    return out.astype(np.float32)



# revision 27
# speedup vs baseline: 231.5336x; 1.0982x over previous
"""Trainium2 Bass kernel for nn_MixingNetwork (QMIX-style mixer).

Math (per sample b):
  hid_w = (state @ W_hw).reshape(H, N); out_w = state @ W_ow; hid_b = state @ W_hb
  h     = elu(hid_w @ q + hid_b);      q_tot = out_w . h + state @ W_ob (+ biases)

Strategy: pure data parallel over batch (512 samples/core on 8 cores).
v2 arch ([h, b]-oriented): W stationary on the PE; movers = stateT * q[b, n]
built on DVE in bf16 (DVE tensor_tensor runs at 1 elem/cycle -- ~80us, the
second wall after the PE's ~98us); agent sums accumulate in PSUM.

fp8 hybrid: k-tile 3 (1/4 of the contraction) runs as fp8e4m3 DoubleRow,
pairing the 2 agents of an f-chunk into one matmul (2x PE rate). More fp8
fails the 2e-2 gate: full-fp8 measures 3.4e-2, 1/4 measures 1.73e-2
(e4m3's 3-bit mantissa on both operands; one-side-exact splits cost the
pairing back). The fp8 movers are cast bf16->fp8 on the otherwise-idle ACT
engine (Copy, scale=1/4; direct fp8-out TT on DVE is 2x slower and makes
DVE the critical path). W8 carries the balancing 4x on the host.
All biases are folded in as rank-1 / small matmuls accumulated in PSUM;
the elu+1 compensation column is host-folded into W_ob (QP: one pass).
"""

import numpy as np
import ml_dtypes

B, N_AGENTS, HIDDEN, STATE_DIM = 4096, 64, 256, 512
N_CORES = 8
BS = B // N_CORES          # samples per core
NBT = BS // 128            # b-tiles per core
KT = STATE_DIM // 128      # k-tiles over state dim
FCHUNK = 512               # columns of W per PSUM chunk
NF = HIDDEN * N_AGENTS // FCHUNK   # 32 chunks
NPC = FCHUNK // HIDDEN     # agents (n) per chunk = 2
GROUP = 8                  # f-chunks per partial-reduce group
NG = NF // GROUP           # groups

_CACHE = {}

# build-time tuning knobs (A/B testing)
CFG = {
    "dve_every": 4,      # every Nth scale op on DVE (0 = all on ACT)
    "sync_w_dma": True,  # W-chunk DMAs via HWDGE (sync engine)
    "ps_bufs": 4,        # big-psum pool bufs
    "w_bufs": 6,
    "mode": "full",     # "dma" | "mm" | "full" — partial builds for HW bisect
    "arch": "v2",       # v1: scale-from-PSUM + reduce; v2: pre-scaled movers
    "mv_bufs": 24,
    "qrep_sync": True,  # qrep/const broadcast DMAs via HWDGE
    "mv_pair": True,    # one DVE op builds movers for both agents of a chunk
    "fp8k": 1,          # of the 4 k-tiles, how many (from the top) run as
                        # fp8e4m3 DoubleRow (2 agents/MM). err ~1.7e-2 at 1.
    "fp8_s": 4.0,       # balance scale: W*s on host, state/s on device
    "mv8_via": "act",   # "act": DVE builds bf16 mover, ACT casts to fp8
                        # (keeps DVE at its bf16 rate); "dve": direct fp8 TT
    "korder": (0, 3, 1, 2),  # emission order of k-tiles within an f-chunk:
                        # DR matmuls mid-chunk hide their longer LDWEIGHTS
    "w8_gp": False,     # w8t DMAs on the Pool queue (measured: neutral)
    "swil": False,      # DoubleRowSwInterleave: host pre-interleaves W pairs
                        # (contiguous LDWEIGHTS, no HW reversal penalty)
}


def _emit_body(nc, tc, ctx, tile, mybir, dram, pools):
    bass = pools["bass"]
    bf16 = mybir.dt.bfloat16
    f32 = mybir.dt.float32
    AX = mybir.AxisListType
    ALU = mybir.AluOpType
    ACTF = mybir.ActivationFunctionType

    stateT, q, qT, w_perm, w_small, bmat, bias_small, out = dram
    consts, wpool, spool, papool, hpool, pspool, smpool = (
        pools["consts"], pools["w"], pools["s"], pools["pa"], pools["h"],
        pools["ps"], pools["sm"],
    )

    # ---- constants into SBUF (emitted per rep; cheap) ----
    stateT_sb = consts.tile([128, KT, BS], bf16, tag="stateT")
    nc.sync.dma_start(stateT_sb[:], stateT.rearrange("(k p) b -> p k b", p=128))
    q_sb = consts.tile([128, NBT, N_AGENTS], f32, tag="q")
    nc.gpsimd.dma_start(q_sb[:], q.rearrange("(t p) n -> p t n", p=128))
    wsm_sb = consts.tile([128, KT, HIDDEN * 2 + 2], bf16, tag="wsm")
    nc.gpsimd.dma_start(wsm_sb[:], w_small.rearrange("(k p) c -> p k c", p=128))
    qT_sb = consts.tile([N_AGENTS, BS], bf16, tag="qT")
    nc.gpsimd.dma_start(qT_sb[:], qT[:, :])
    bmat_sb = consts.tile([N_AGENTS, HIDDEN], bf16, tag="bmat")
    nc.gpsimd.dma_start(bmat_sb[:], bmat[:, :])
    bias_sb = consts.tile([1, HIDDEN * 2 + 2], bf16, tag="bias")
    nc.gpsimd.dma_start(bias_sb[:], bias_small[:, :])
    ones_sb = consts.tile([1, 128], bf16, tag="ones")
    nc.vector.memset(ones_sb[:], 1.0)

    partials = [papool.tile([128, NG, HIDDEN], f32, tag=f"pa{bt}", name=f"pa{bt}")
                for bt in range(NBT)]
    S = [None] * NBT

    # ---- big contraction: G = stateT.T @ W_perm, scaled by q, reduced over n ----
    # fraction of scale ops routed to DVE (rest on ACT) to balance engines
    ndve = 0
    for f in range(NF):
        wt = wpool.tile([128, KT, FCHUNK], bf16, tag="w")
        dma_eng = nc.sync if CFG["sync_w_dma"] else nc.gpsimd
        dma_eng.dma_start(
            wt[:],
            w_perm.rearrange("(k p) n -> p k n", p=128)[:, :, f * FCHUNK:(f + 1) * FCHUNK],
        )
        g, pos = divmod(f, GROUP)
        for bt in range(NBT):
            if pos == 0:
                S[bt] = spool.tile([128, NPC * GROUP, HIDDEN], bf16, tag=f"S{bt}",
                                   name=f"S{bt}")
            if CFG["mode"] == "dma":
                continue
            ps = pspool.tile([128, FCHUNK], f32, tag="ps")
            bsl = slice(bt * 128, (bt + 1) * 128)
            for k in range(KT):
                nc.tensor.matmul(
                    ps[:], stateT_sb[:, k, bsl], wt[:, k, :],
                    start=(k == 0), stop=(k == KT - 1),
                )
            for j in range(NPC if CFG["mode"] == "full" else 0):
                n_local = NPC * pos + j
                n_glob = NPC * f + j
                ndve += 1
                if CFG["dve_every"] and ndve % CFG["dve_every"] == 0:
                    nc.vector.tensor_scalar_mul(
                        S[bt][:, n_local, :], ps[:, j * HIDDEN:(j + 1) * HIDDEN],
                        q_sb[:, bt, n_glob:n_glob + 1],
                    )
                else:
                    nc.scalar.activation(
                        S[bt][:, n_local, :], ps[:, j * HIDDEN:(j + 1) * HIDDEN],
                        ACTF.Copy, scale=q_sb[:, bt, n_glob:n_glob + 1],
                    )
            if pos == GROUP - 1 and CFG["mode"] == "full":
                nc.vector.tensor_reduce(
                    partials[bt][:, g, :],
                    S[bt][:].rearrange("p n h -> p h n"),
                    axis=AX.X, op=ALU.add,
                )

    # ---- per-b-tile tail: hypernet biases + small matmuls + ELU + final dot ----
    for bt in range(NBT if CFG["mode"] == "full" else 0):
        bsl = slice(bt * 128, (bt + 1) * 128)
        hs = smpool.tile([128, HIDDEN], f32, tag="hsum")
        for k in range(KT):
            nc.tensor.matmul(hs[:], stateT_sb[:, k, bsl], wsm_sb[:, k, 0:HIDDEN],
                             start=(k == 0), stop=False)
        nc.tensor.matmul(hs[:], qT_sb[:, bsl], bmat_sb[:], start=False, stop=False)
        nc.tensor.matmul(hs[:], ones_sb[:, 0:128], bias_sb[:, 0:HIDDEN],
                         start=False, stop=True)

        ow = smpool.tile([128, HIDDEN], f32, tag="ow")
        for k in range(KT):
            nc.tensor.matmul(ow[:], stateT_sb[:, k, bsl],
                             wsm_sb[:, k, HIDDEN:2 * HIDDEN],
                             start=(k == 0), stop=False)
        nc.tensor.matmul(ow[:], ones_sb[:, 0:128], bias_sb[:, HIDDEN:2 * HIDDEN],
                         start=False, stop=True)

        ob = smpool.tile([128, 1], f32, tag="ob")
        for k in range(KT):
            nc.tensor.matmul(ob[:], stateT_sb[:, k, bsl],
                             wsm_sb[:, k, 2 * HIDDEN:2 * HIDDEN + 1],
                             start=(k == 0), stop=False)
        nc.tensor.matmul(ob[:], ones_sb[:, 0:128], bias_sb[:, 2 * HIDDEN:2 * HIDDEN + 1],
                         start=False, stop=True)

        hpre = hpool.tile([128, HIDDEN], f32, tag="hpre")
        nc.vector.tensor_reduce(
            hpre[:], partials[bt][:].rearrange("p g h -> p h g"), axis=AX.X, op=ALU.add,
        )
        hp = hpool.tile([128, HIDDEN], f32, tag="hp")
        nc.vector.tensor_add(hp[:], hpre[:], hs[:])
        # elu(x) = max(x,0) + exp(min(x,0)) - 1
        t0 = hpool.tile([128, HIDDEN], f32, tag="t0")
        nc.vector.tensor_scalar_min(t0[:], hp[:], 0.0)
        ex = hpool.tile([128, HIDDEN], f32, tag="ex")
        nc.scalar.activation(ex[:], t0[:], ACTF.Exp)
        t1 = hpool.tile([128, HIDDEN], f32, tag="t1")
        nc.vector.tensor_scalar_max(t1[:], hp[:], 0.0)
        h2 = hpool.tile([128, HIDDEN], f32, tag="h2")
        nc.vector.tensor_add(h2[:], t1[:], ex[:])
        h3 = hpool.tile([128, HIDDEN], f32, tag="h3")
        nc.vector.tensor_scalar_add(h3[:], h2[:], -1.0)

        scr = hpool.tile([128, HIDDEN], f32, tag="scr")
        nc.vector.tensor_mul(scr[:], h3[:], ow[:])
        qts = hpool.tile([128, 1], f32, tag="qts")
        nc.vector.tensor_reduce(qts[:], scr[:], axis=AX.X, op=ALU.add)
        qt = hpool.tile([128, 1], f32, tag="qt")
        nc.vector.tensor_add(qt[:], qts[:], ob[:, 0:1])
        nc.gpsimd.dma_start(out[bsl, :], qt[:])


def _emit_body_v2(nc, tc, ctx, tile, mybir, dram, pools):
    """[h, b]-oriented pipeline: W stationary, movers = stateT * q[b, n] built
    on DVE (bf16 SBUF 2x); agent-sum accumulates in PSUM; final dot = ones
    matmul over h partitions. ACT nearly idle; PE-bound."""
    import concourse.bass as bass
    bf16 = mybir.dt.bfloat16
    f32 = mybir.dt.float32
    fp8 = mybir.dt.float8e4
    AX = mybir.AxisListType
    ALU = mybir.AluOpType
    ACTF = mybir.ActivationFunctionType
    H2 = HIDDEN // 128              # h-tiles (2)
    KF8 = KT - CFG["fp8k"]          # k-tiles >= KF8 run fp8 DoubleRow

    stateT, q, qT, w_perm, w_small, bmat, bias_small, w8, out = dram
    consts, wpool, mvpool, hpool, pspool = (
        pools["consts"], pools["w"], pools["mv"], pools["h"], pools["ps"])

    dma = (nc.sync if CFG["sync_w_dma"] else nc.gpsimd).dma_start
    qdma = (nc.sync if CFG["qrep_sync"] else nc.gpsimd).dma_start

    # ---- constants; stateT split per k-tile so the first slice lands fast.
    # stateT lives in a bufs=2 pool: it is read until the very last matmul
    # of a rep, so rep N+1's reload must double-buffer or the PE stalls
    # ~2.5us at every rep boundary.
    stateT_sb = pools["stp"].tile([128, KT, BS], bf16, tag="stateT")
    stateT_r = stateT.rearrange("(k p) b -> p k b", p=128)
    for k in range(KT):
        dma(stateT_sb[:, k, :], stateT_r[:, k, :])
    wsm_sb = consts.tile([128, KT, HIDDEN * 2 + 2], bf16, tag="wsm")
    nc.gpsimd.dma_start(wsm_sb[:], w_small.rearrange("(k p) c -> p k c", p=128))
    qT_sb = consts.tile([N_AGENTS, BS], bf16, tag="qT")
    nc.gpsimd.dma_start(qT_sb[:], qT[:, :])
    bmat_sb = consts.tile([N_AGENTS, HIDDEN], bf16, tag="bmat")
    nc.gpsimd.dma_start(bmat_sb[:], bmat[:, :])
    bias_sb = consts.tile([1, HIDDEN * 2 + 2], bf16, tag="bias")
    nc.gpsimd.dma_start(bias_sb[:], bias_small[:, :])
    ones_row = consts.tile([1, BS], bf16, tag="ones_row")
    nc.vector.memset(ones_row[:], 1.0)
    ones_col = consts.tile([128, 1], bf16, tag="ones_col")
    nc.vector.memset(ones_col[:], 1.0)

    NQG = 8
    NQTILES = N_AGENTS // NQG
    qrep = [consts.tile([128, NQG, BS], bf16, tag=f"qrep{g}", name=f"qrep{g}")
            for g in range(NQTILES)]

    Y = [pools["psy"].tile([128, BS], f32, tag=f"Y{ht}", name=f"Y{ht}")
         for ht in range(H2)]
    OW = [pspool.tile([128, BS], f32, tag=f"OW{ht}", name=f"OW{ht}") for ht in range(H2)]
    QP = pspool.tile([1, BS], f32, tag="QP")

    # ---- small matmuls first: they run during the DMA ramp-up while the
    # first W chunks stream in. Y accumulation group OPENS here (start=True)
    # and is closed by the last big-loop matmul.
    for ht in range(H2):
        hsl = slice(ht * 128, (ht + 1) * 128)
        for k in range(KT):
            nc.tensor.matmul(Y[ht][:], wsm_sb[:, k, hsl], stateT_sb[:, k, :],
                             start=(k == 0), stop=False, skip_group_check=True)
        nc.tensor.matmul(Y[ht][:], bmat_sb[:, hsl], qT_sb[:, :],
                         start=False, stop=False, skip_group_check=True)
        nc.tensor.matmul(Y[ht][:], bias_sb[:, hsl], ones_row[:, :],
                         start=False, stop=False, skip_group_check=True)
        for k in range(KT):
            nc.tensor.matmul(OW[ht][:], wsm_sb[:, k, HIDDEN + ht * 128:HIDDEN + (ht + 1) * 128],
                             stateT_sb[:, k, :],
                             start=(k == 0), stop=False, skip_group_check=True)
        nc.tensor.matmul(OW[ht][:], bias_sb[:, HIDDEN + ht * 128:HIDDEN + (ht + 1) * 128],
                         ones_row[:, :], start=False, stop=True, skip_group_check=True)
    # w_small col 2H already holds W_ob - sum_h W_ow (host-folded); the
    # -sum_h out_w term compensates using elu+1 in the tail
    for k in range(KT):
        nc.tensor.matmul(QP[:], wsm_sb[:, k, 2 * HIDDEN:2 * HIDDEN + 1],
                         stateT_sb[:, k, :], start=(k == 0), stop=False,
                         skip_group_check=True)
    nc.tensor.matmul(QP[:], bias_sb[:, 2 * HIDDEN:2 * HIDDEN + 1], ones_row[:, :],
                     start=False, stop=False, skip_group_check=True)

    # ---- fp8 k-tiles: state slices pre-scaled by 1/s (W carries s on host)
    st8 = []
    if CFG["fp8k"] and CFG["mv8_via"] == "dve":
        for kk in range(CFG["fp8k"]):
            t = hpool.tile([128, BS], bf16, tag=f"st8_{kk}", name=f"st8_{kk}")
            nc.vector.tensor_scalar_mul(t[:], stateT_sb[:, KF8 + kk, :],
                                        1.0 / CFG["fp8_s"])
            st8.append(t)

    # ---- big streamed contraction; qrep slices interleave with the W stream
    w_perm_r = w_perm.rearrange("(k p) n -> p k n", p=128)
    w8_r = w8.rearrange("p (f c) -> p f c", f=NF) if w8 is not None else None
    for f in range(NF):
        if f % (NF // NQTILES) == 0:
            g = f // (NF // NQTILES)
            qsrc = bass.AP(
                tensor=qT.tensor, offset=g * NQG * BS,
                ap=[[0, 128], [BS, NQG], [1, BS]],
            )
            qdma(qrep[g][:], qsrc)
        if KF8 > 0:
            wt = wpool.tile([128, KF8, FCHUNK], bf16, tag="w")
            dma(wt[:], w_perm_r[:, 0:KF8, f * FCHUNK:(f + 1) * FCHUNK])
        if CFG["fp8k"]:
            w8t = wpool.tile([128, CFG["fp8k"], NPC, HIDDEN], fp8, tag="w8")
            w8dma = nc.gpsimd.dma_start if CFG["w8_gp"] else dma
            w8dma(w8t[:].rearrange("p kk j h -> p (kk j h)"), w8_r[:, f, :])
        n0 = NPC * f
        korder = [k for k in CFG["korder"] if k < KT]
        for k in korder:
            last = (f == NF - 1 and k == korder[-1])
            qr = qrep[n0 // NQG][:, n0 % NQG:n0 % NQG + NPC, :]
            if k >= KF8:
                kk = k - KF8
                mv8 = mvpool.tile([128, NPC, BS], fp8, tag="mv8")
                if CFG["mv8_via"] == "act":
                    mvb = mvpool.tile([128, NPC, BS], bf16, tag="mv")
                    st_rep = bass.AP(
                        tensor=stateT_sb.tensor,
                        offset=stateT_sb[:, k, :].offset,
                        ap=[stateT_sb[:].ap[0], [0, NPC], [1, BS]],
                    )
                    nc.vector.tensor_mul(mvb[:], st_rep, qr)
                    nc.scalar.activation(
                        mv8[:].rearrange("p j b -> p (j b)"),
                        mvb[:].rearrange("p j b -> p (j b)"),
                        ACTF.Copy, scale=1.0 / CFG["fp8_s"],
                    )
                else:
                    st_rep = bass.AP(
                        tensor=st8[kk].tensor, offset=st8[kk][:].offset,
                        ap=[st8[kk][:].ap[0], [0, NPC], [1, BS]],
                    )
                    nc.vector.tensor_mul(mv8[:], st_rep, qr)
                for ht in range(H2):
                    if CFG["swil"]:
                        lhsT = w8t[:, kk, ht, :]
                        pm = mybir.MatmulPerfMode.DoubleRowSwInterleave
                    else:
                        lhsT = w8t[:, kk, :, ht * 128:(ht + 1) * 128]
                        pm = mybir.MatmulPerfMode.DoubleRow
                    nc.tensor.matmul(
                        Y[ht][:], lhsT,
                        mv8[:], start=False, stop=last and ht == H2 - 1,
                        perf_mode=pm, skip_group_check=True,
                    )
                continue
            mv2 = mvpool.tile([128, NPC, BS], bf16, tag="mv")
            st_rep = bass.AP(
                tensor=stateT_sb.tensor, offset=stateT_sb[:, k, :].offset,
                ap=[stateT_sb[:].ap[0], [0, NPC], [1, BS]],
            )
            nc.vector.tensor_mul(mv2[:], st_rep, qr)
            for j in range(NPC):
                for ht in range(H2):
                    nc.tensor.matmul(
                        Y[ht][:], wt[:, k, j * HIDDEN + ht * 128:j * HIDDEN + (ht + 1) * 128],
                        mv2[:, j, :], start=False,
                        stop=last and j == NPC - 1, skip_group_check=True,
                    )

    # ---- tail: elu + dot with out_w^T, h-reduction via ones matmul.
    # Split along b so the chain pipelines in smaller quanta.
    NBH = CFG.get("nbh", 1)
    BH = BS // NBH
    for ht in range(H2):
        for hb in range(NBH):
            bsl = slice(hb * BH, (hb + 1) * BH)
            t0 = hpool.tile([128, BH], f32, tag="t0")
            nc.vector.tensor_scalar_min(t0[:], Y[ht][:, bsl], 0.0)
            ex = hpool.tile([128, BH], f32, tag="ex")
            nc.scalar.activation(ex[:], t0[:], ACTF.Exp)
            rl = hpool.tile([128, BH], f32, tag="rl")
            nc.scalar.activation(rl[:], Y[ht][:, bsl], ACTF.Relu)
            h3 = hpool.tile([128, BH], f32, tag="h3")
            nc.vector.tensor_add(h3[:], rl[:], ex[:])
            mT = hpool.tile([128, BH], bf16, tag="mT")
            nc.vector.tensor_mul(mT[:], h3[:], OW[ht][:, bsl])
            nc.tensor.matmul(QP[:, bsl], ones_col[:, 0:1], mT[:],
                             start=False, stop=(ht == H2 - 1 and hb == NBH - 1),
                             skip_group_check=True)
    qrow = hpool.tile([1, BS], f32, tag="qrow")
    nc.scalar.activation(qrow[:], QP[:], ACTF.Copy)
    nc.gpsimd.dma_start(out[:, :], qrow[:])


def build_module(reps=1, loop_reps=1):
    """Build and compile the per-core Bass module. reps>1 repeats the whole
    computation in one NEFF (for timing)."""
    from contextlib import ExitStack
    import concourse.bass as bass
    import concourse.tile as tile
    from concourse import bacc, mybir

    bf16 = mybir.dt.bfloat16
    f32 = mybir.dt.float32

    nc = bacc.Bacc("TRN2", target_bir_lowering=False)
    stateT = nc.dram_tensor("stateT", [STATE_DIM, BS], bf16, kind="ExternalInput").ap()
    q = nc.dram_tensor("q", [BS, N_AGENTS], f32, kind="ExternalInput").ap()
    qT = nc.dram_tensor("qT", [N_AGENTS, BS], bf16, kind="ExternalInput").ap()
    w_perm = nc.dram_tensor("w_perm", [STATE_DIM, HIDDEN * N_AGENTS], bf16,
                            kind="ExternalInput").ap()
    w_small = nc.dram_tensor("w_small", [STATE_DIM, HIDDEN * 2 + 2], bf16,
                             kind="ExternalInput").ap()
    bmat = nc.dram_tensor("bmat", [N_AGENTS, HIDDEN], bf16, kind="ExternalInput").ap()
    bias_small = nc.dram_tensor("bias_small", [1, HIDDEN * 2 + 2], bf16,
                                kind="ExternalInput").ap()
    w8 = None
    if CFG["arch"] == "v2" and CFG["fp8k"]:
        w8 = nc.dram_tensor(
            "w8", [128, NF * CFG["fp8k"] * NPC * HIDDEN], mybir.dt.float8e4,
            kind="ExternalInput").ap()
    if CFG["arch"] == "v2":
        out = nc.dram_tensor("out", [1, BS], f32, kind="ExternalOutput").ap()
        dram = (stateT, q, qT, w_perm, w_small, bmat, bias_small, w8, out)
    else:
        out = nc.dram_tensor("out", [BS, 1], f32, kind="ExternalOutput").ap()
        dram = (stateT, q, qT, w_perm, w_small, bmat, bias_small, out)

    with tile.TileContext(nc) as tc:
        with ExitStack() as ctx:
            if CFG["arch"] == "v2":
                pools = {
                    "bass": bass,
                    "consts": ctx.enter_context(tc.tile_pool(name="consts", bufs=1)),
                    "stp": ctx.enter_context(tc.tile_pool(name="stp", bufs=2)),
                    "w": ctx.enter_context(tc.tile_pool(name="w", bufs=CFG["w_bufs"])),
                    "mv": ctx.enter_context(tc.tile_pool(name="mv", bufs=CFG["mv_bufs"])),
                    "h": ctx.enter_context(tc.tile_pool(name="h", bufs=2)),
                    "ps": ctx.enter_context(tc.tile_pool(name="ps", bufs=1, space="PSUM")),
                    # Y double-buffered: rep N+1's first matmuls must not wait
                    # for rep N's tail to drain Y out of PSUM (7/8 banks used)
                    "psy": ctx.enter_context(tc.tile_pool(name="psy", bufs=2, space="PSUM")),
                }
                emit = _emit_body_v2
            else:
                pools = {
                    "bass": bass,
                    "consts": ctx.enter_context(tc.tile_pool(name="consts", bufs=1)),
                    "w": ctx.enter_context(tc.tile_pool(name="w", bufs=CFG["w_bufs"])),
                    "s": ctx.enter_context(tc.tile_pool(name="s", bufs=2)),
                    "pa": ctx.enter_context(tc.tile_pool(name="pa", bufs=1)),
                    "h": ctx.enter_context(tc.tile_pool(name="h", bufs=2)),
                    "ps": ctx.enter_context(tc.tile_pool(name="ps", bufs=CFG["ps_bufs"], space="PSUM")),
                    "sm": ctx.enter_context(tc.tile_pool(name="sm", bufs=1, space="PSUM")),
                }
                emit = _emit_body
            if loop_reps > 1:
                with tc.For_i(0, loop_reps, 1,
                              hint_engines=(mybir.EngineType.PE,
                                            mybir.EngineType.DVE)):
                    for _ in range(reps):
                        emit(nc, tc, ctx, tile, mybir, dram, pools)
            else:
                for _ in range(reps):
                    emit(nc, tc, ctx, tile, mybir, dram, pools)
    nc.compile()
    return nc


def make_in_maps(q_values, state_representation, W_hw, b_hw, W_ow, b_ow, W_hb,
                 b_hb, W_ob, b_ob):
    bf16 = ml_dtypes.bfloat16
    q = np.asarray(q_values, dtype=np.float32).reshape(B, N_AGENTS)
    st = np.asarray(state_representation, dtype=np.float32)
    W_hw = np.asarray(W_hw, dtype=np.float32)
    # permute columns of W_hw from (h, n) to (n, h) order
    w_perm = np.ascontiguousarray(
        W_hw.reshape(STATE_DIM, HIDDEN, N_AGENTS).transpose(0, 2, 1)
        .reshape(STATE_DIM, HIDDEN * N_AGENTS)).astype(bf16)
    W_ow32 = np.asarray(W_ow, np.float32)
    ob_col = np.asarray(W_ob, np.float32)
    ob_bias = np.asarray(b_ob, np.float32)
    b_ow32 = np.asarray(b_ow, np.float32)
    if CFG["arch"] == "v2":
        # fold the elu+1 compensation (-sum_h W_ow / -sum_h b_ow) into the
        # W_ob column so QP needs one contraction instead of two
        ob_col = ob_col - W_ow32.sum(axis=1, keepdims=True)
        ob_bias = ob_bias - b_ow32.sum(keepdims=True)
    w_small = np.ascontiguousarray(np.concatenate(
        [np.asarray(W_hb, np.float32), W_ow32, ob_col,
         -W_ow32.sum(axis=1, keepdims=True)], axis=1)).astype(bf16)
    bmat = np.ascontiguousarray(
        np.asarray(b_hw, np.float32).reshape(HIDDEN, N_AGENTS).T).astype(bf16)
    bias_small = np.concatenate(
        [np.asarray(b_hb, np.float32), b_ow32, ob_bias,
         -b_ow32.sum(keepdims=True)]).reshape(1, HIDDEN * 2 + 2).astype(bf16)
    w8a = None
    if CFG["arch"] == "v2" and CFG["fp8k"]:
        fp8k, kf8 = CFG["fp8k"], KT - CFG["fp8k"]
        wp3 = W_hw.reshape(STATE_DIM, HIDDEN, N_AGENTS).transpose(0, 2, 1)
        # [kk, p, f, j, h] -> [p, (f, kk, j, h)]
        arr = wp3[kf8 * 128:].reshape(fp8k, 128, NF, NPC, HIDDEN)
        if CFG["swil"]:
            # flat col = 2*(127 - h_local) + j, per (kk, ht): the HW reads
            # SW-interleaved weights contiguously (pairs adjacent, reversed)
            a6 = arr.reshape(fp8k, 128, NF, NPC, HIDDEN // 128, 128)
            a6 = a6.transpose(1, 2, 0, 4, 5, 3)[:, :, :, :, ::-1, :]
            w8a = np.ascontiguousarray(
                a6 * CFG["fp8_s"]
            ).astype(ml_dtypes.float8_e4m3).reshape(128, -1)
        else:
            w8a = np.ascontiguousarray(
                arr.transpose(1, 2, 0, 3, 4) * CFG["fp8_s"]
            ).astype(ml_dtypes.float8_e4m3).reshape(128, -1)
    in_maps = []
    for c in range(N_CORES):
        sl = slice(c * BS, (c + 1) * BS)
        m = {
            "stateT": np.ascontiguousarray(st[sl].T).astype(bf16),
            "q": np.ascontiguousarray(q[sl]),
            "qT": np.ascontiguousarray(q[sl].T).astype(bf16),
            "w_perm": w_perm,
            "w_small": w_small,
            "bmat": bmat,
            "bias_small": bias_small,
        }
        if w8a is not None:
            m["w8"] = w8a
        in_maps.append(m)
    return in_maps


def kernel(**inputs):
    from concourse.bass_utils import run_bass_kernel_spmd

    if "nc" not in _CACHE:
        _CACHE["nc"] = build_module()
    nc = _CACHE["nc"]
    in_maps = make_in_maps(**inputs)
    res = run_bass_kernel_spmd(nc, in_maps, core_ids=list(range(N_CORES)))
    if CFG["arch"] == "v2":
        out = np.concatenate(
            [res.results[c]["out"][0] for c in range(N_CORES)]).reshape(B, 1)
    else:
        out = np.concatenate([res.results[c]["out"] for c in range(N_CORES)], axis=0)
    return out.astype(np.float32)

